# revision 1
# baseline (speedup 1.0000x reference)
"""Trainium2 distributed Bass kernel for the hierarchical GNN encoder.

Strategy (8 NeuronCores, SPMD):
  - Shard the S=8192 subgraphs contiguously: 1024 subgraphs (=32768 flat rows,
    512 original nodes) per core.  Intra edges are subgraph-local so each
    core's intra edges are fully local.
  - h lives in DRAM row-major [32768, H] bf16 per core.
  - Intra/global GINE aggregation: edges are sorted by destination on the
    host and packed into 128-edge tiles such that no destination row
    straddles a tile and each tile stays inside one 128-row "window" of the
    output.  Per tile: dma_gather h[src] and bond[typ] (SWDGE), relu(add),
    build the dst one-hot with one DVE is_equal against an iota tile, then a
    PE matmul accumulates the window's agg rows in PSUM.  No indirect
    scatter is needed (plain per-window stores), so there are no RMW races.
  - MLPs run on PE with weights stationary; orientation alternates between
    row-major and feature-major via DMA-transpose loads (bf16).
  - BatchNorm batch stats are all-reduced ([H,2] per norm); h_node uses a
    local pairwise mean + AllGather; the global agg uses ReduceScatter; the
    final pooled [64,H] output is all-reduced.
"""

import math
import os
import sys

sys.path.insert(0, "/opt/trn_rl_repo")

import numpy as np
import ml_dtypes

from concourse import bacc, bass, mybir, tile
from concourse.bass_utils import run_bass_kernel_spmd

P = 128
H = 128
L = 4
NCORES = 8
NG = 64
Nn = 4096
M_SUB = 2          # subgraphs per node
K_SUB = 32         # nodes per subgraph
S_TOT = Nn * M_SUB
SK = S_TOT * K_SUB
ROWS = SK // NCORES          # 32768 flat rows per core
SUBS = S_TOT // NCORES       # 1024 subgraphs per core
NODES = Nn // NCORES         # 512 nodes per core
NWIN = ROWS // P             # 256 agg windows per core
NWIN_G = Nn // P             # 32 global agg windows
F32 = mybir.dt.float32
BF16 = mybir.dt.bfloat16
I16 = mybir.dt.int16
I32 = mybir.dt.int32
AF = mybir.ActivationFunctionType
ALU = mybir.AluOpType
BF = ml_dtypes.bfloat16


# ----------------------------------------------------------------------------
# Host-side edge packing
# ----------------------------------------------------------------------------

def pack_edges(src, dst, typ, n_rows, n_win, pad_typ=16):
    """Sort edges by dst and pack into 128-edge tiles.

    Each tile's edges all target one 128-row window and no dst row straddles
    tiles.  Returns (src_t, typ_t, dstrel_t) with shape [ntiles, 128] and the
    window id of each tile.  Padding slots: src=0, typ=pad_typ, dstrel=-1.
    """
    order = np.argsort(dst, kind="stable")
    src, dst, typ = src[order], dst[order], typ[order]
    n = len(dst)
    tiles_src, tiles_typ, tiles_rel, tiles_win = [], [], [], []
    cur_s, cur_t, cur_r = [], [], []
    cur_win = -1

    def flush():
        nonlocal cur_s, cur_t, cur_r
        if cur_win < 0:
            return
        pad = P - len(cur_s)
        tiles_src.append(np.array(cur_s + [0] * pad, np.int64))
        tiles_typ.append(np.array(cur_t + [pad_typ] * pad, np.int64))
        tiles_rel.append(np.array(cur_r + [-1.0] * pad, np.float64))
        tiles_win.append(cur_win)
        cur_s, cur_t, cur_r = [], [], []

    i = 0
    while i < n:
        j = i
        d = dst[i]
        while j < n and dst[j] == d:
            j += 1
        run = j - i
        w = d // P
        assert run <= P, f"dst run {run} exceeds tile"
        if cur_win >= 0 and (w != cur_win or len(cur_s) + run > P):
            flush()
        cur_win = w
        cur_s += list(src[i:j])
        cur_t += list(typ[i:j])
        cur_r += [float(d - w * P)] * run
        i = j
    flush()
    return (np.array(tiles_src), np.array(tiles_typ),
            np.array(tiles_rel), np.array(tiles_win, np.int64))


def layout_windows(t_src, t_typ, t_rel, t_win, n_win, tpw):
    """Arrange packed tiles into a dense [n_win, tpw, 128] layout."""
    src = np.zeros((n_win, tpw, P), np.int64)
    typ = np.full((n_win, tpw, P), 16, np.int64)
    rel = np.full((n_win, tpw, P), -1.0, np.float64)
    fill = np.zeros(n_win, np.int64)
    for t in range(len(t_win)):
        w = t_win[t]
        j = fill[w]
        assert j < tpw
        src[w, j] = t_src[t]
        typ[w, j] = t_typ[t]
        rel[w, j] = t_rel[t]
        fill[w] += 1
    return src, typ, rel


def wrap16(idx):
    """[n] int -> [16, n//16] int16 wrapped layout for dma_gather."""
    n = len(idx)
    assert n % 16 == 0
    return np.ascontiguousarray(idx.reshape(n // 16, 16).T.astype(np.int16))


# ----------------------------------------------------------------------------
# Device program
# ----------------------------------------------------------------------------

def build_program(tpw, tpw_g, eps_l, eps_g):
    nc = bacc.Bacc(None, target_bir_lowering=False, debug=True)

    def inp(name, shape, dtype):
        return nc.declare_dram_parameter(name, list(shape), dtype, isOutput=False)

    # weights / tables
    atom = inp("atom", [P, H], BF16)
    bond = inp("bond", [32, H], BF16)           # 16 real rows, row16 = -1e4
    rwse_lt = inp("rwse_lt", [16, Nn], BF16)    # rwse^T as lhsT tiles
    rwse_w = inp("rwse_w", [16, H], BF16)
    rwse_brep = inp("rwse_brep", [P, H], F32)
    wl1 = inp("wl1", [L * H, H], BF16)
    wl2 = inp("wl2", [L * H, H], BF16)
    gw1 = inp("gw1", [L * H, H], BF16)
    gw2 = inp("gw2", [L * H, H], BF16)
    bcw = inp("bcw", [L * H, H], BF16)
    cw1t = inp("cw1t", [L * H, H], BF16)        # cat_w1 top half
    cw1b = inp("cw1b", [L * H, H], BF16)        # cat_w1 bottom half
    cw2 = inp("cw2", [L * H, H], BF16)
    bias_cols = inp("bias_cols", [P, 8 * L], F32)
    # per layer: [b1, b2, gb1, gb2, catb1, bng, bnb_, gbng] packed columns;
    # plus a second tensor for the rest
    bias2_cols = inp("bias2_cols", [P, 4 * L], F32)  # [gbnb, lng?, ...] see host
    cb2rep = inp("cb2rep", [L * P, H], F32)     # cat_b2 replicated per layer
    lngrep = inp("lngrep", [L * P, H], F32)
    lnbrep = inp("lnbrep", [L * P, H], F32)
    iota_rep = inp("iota_rep", [P, P], F32)
    validf = inp("validf", [P, NWIN], F32)
    wpool = inp("wpool", [P, NWIN * NG], BF16)
    # index tensors
    x32 = inp("x32", [P, NWIN], I32)
    n32 = inp("n32", [P, NWIN], I32)
    isrc = inp("isrc", [P, NWIN * tpw], I32)
    toh = inp("toh", [32, NWIN * tpw * P], BF16)
    idst = inp("idst", [P, NWIN * tpw], F32)
    gsrc = inp("gsrc", [P, NWIN_G * tpw_g], I32)
    gtoh = inp("gtoh", [32, NWIN_G * tpw_g * P], BF16)
    gdst = inp("gdst", [P, NWIN_G * tpw_g], F32)

    out_ext = nc.declare_dram_parameter("out", [NG, H], F32, isOutput=True)

    # internal DRAM
    h_d = nc.dram_tensor("h_d", [ROWS, H], BF16)
    h_pm = nc.dram_tensor("h_pm", [P, NWIN, H], BF16)
    hlin_d = nc.dram_tensor("hlin_d", [ROWS, H], BF16)
    hrT_d = nc.dram_tensor("hrT_d", [H, ROWS], BF16)
    r_d = nc.dram_tensor("r_d", [Nn, H], BF16)
    hn_d = nc.dram_tensor("hn_d", [NODES, H], BF16)
    hnfull_d = nc.dram_tensor("hnfull_d", [Nn, H], BF16)
    hlinN_d = nc.dram_tensor("hlinN_d", [NODES, H], BF16)
    aggN_d = nc.dram_tensor("aggN_d", [Nn, H], BF16)
    aggN_rs = nc.dram_tensor("aggN_rs", [NODES, H], BF16)
    stat_in = nc.dram_tensor("stat_in", [P, 2], F32)
    stat_out = nc.dram_tensor("stat_out", [P, 2], F32)
    statg_in = nc.dram_tensor("statg_in", [P, 2], F32)
    statg_out = nc.dram_tensor("statg_out", [P, 2], F32)
    pool_in = nc.dram_tensor("pool_in", [NG, H], F32)
    pool_out = nc.dram_tensor("pool_out", [NG, H], F32)

    RG = [list(range(NCORES))]

    with tile.TileContext(nc) as tc:
        with (
            tc.tile_pool(name="const", bufs=1) as cpool,
            tc.tile_pool(name="sb", bufs=2) as sb,
            tc.tile_pool(name="sbw", bufs=2) as sbw,
            tc.tile_pool(name="ps", bufs=2, space="PSUM") as ps,
            tc.tile_pool(name="ps2", bufs=2, space="PSUM") as ps2,
            tc.tile_pool(name="pspool", bufs=1, space="PSUM") as pspool,
        ):
            # ---- constants resident in SBUF ----
            iota_sb = cpool.tile([P, P], F32)
            nc.sync.dma_start(iota_sb[:], iota_rep[:])
            bias_sb = cpool.tile([P, 8 * L], F32)
            nc.sync.dma_start(bias_sb[:], bias_cols[:])
            bias2_sb = cpool.tile([P, 4 * L], F32)
            nc.sync.dma_start(bias2_sb[:], bias2_cols[:])
            validf_sb = cpool.tile([P, NWIN], F32)
            nc.sync.dma_start(validf_sb[:], validf[:])
            bond_sb = cpool.tile([32, H], BF16)
            nc.sync.dma_start(bond_sb[:], bond[:])

            def gather1(dst_tile, table, idx_col):
                nc.gpsimd.indirect_dma_start(
                    out=dst_tile, out_offset=None, in_=table[:],
                    in_offset=bass.IndirectOffsetOnAxis(ap=idx_col, axis=0))

            # ================= init: R = relu(rwse@rwse_w+b), h0 =============
            brep = sb.tile([P, H], F32, tag="brep")
            nc.sync.dma_start(brep[:], rwse_brep[:])
            for j in range(Nn // P):
                lt = sb.tile([16, P], BF16, tag="rlt")
                nc.sync.dma_start(lt[:], rwse_lt[:, j * P:(j + 1) * P])
                pr = ps.tile([P, H], F32, space="PSUM", tag="ps128")
                ww = sb.tile([16, H], BF16, tag="rww")
                nc.sync.dma_start(ww[:], rwse_w[:])
                nc.tensor.matmul(out=pr[:], lhsT=lt[:], rhs=ww[:],
                                 start=True, stop=True)
                t0 = sb.tile([P, H], F32, tag="rt0")
                nc.vector.tensor_tensor(out=t0[:], in0=pr[:], in1=brep[:],
                                        op=ALU.add)
                t1 = sb.tile([P, H], BF16, tag="rt1")
                nc.scalar.activation(t1[:], t0[:], AF.Relu)
                nc.sync.dma_start(r_d[j * P:(j + 1) * P, :], t1[:])

            x32_sb = cpool.tile([P, NWIN], I32)
            nc.sync.dma_start(x32_sb[:], x32[:])
            n32_sb = cpool.tile([P, NWIN], I32)
            nc.sync.dma_start(n32_sb[:], n32[:])
            H0W = 8
            for j in range(NWIN // H0W):
                ga = sb.tile([P, H0W, H], BF16, tag="h0a")
                gr = sb.tile([P, H0W, H], BF16, tag="h0r")
                for cc in range(H0W):
                    gather1(ga[:, cc, :], atom, x32_sb[:, j * H0W + cc:j * H0W + cc + 1])
                    gather1(gr[:, cc, :], r_d, n32_sb[:, j * H0W + cc:j * H0W + cc + 1])
                hsum = sb.tile([P, H0W, H], F32, tag="h0s")
                nc.vector.tensor_tensor(out=hsum[:], in0=ga[:], in1=gr[:],
                                        op=ALU.add)
                hm = sb.tile([P, H0W, H], BF16, tag="h0m")
                vsl = validf_sb[:, j * H0W:(j + 1) * H0W, None]
                nc.vector.tensor_tensor(
                    out=hm[:], in0=hsum[:],
                    in1=vsl.to_broadcast([P, H0W, H]), op=ALU.mult)
                nc.sync.dma_start(h_pm[:, j * H0W:(j + 1) * H0W, :], hm[:])
                for cc in range(H0W):
                    nc.sync.dma_start(
                        h_d[(j * H0W + cc) * P:(j * H0W + cc + 1) * P, :],
                        hm[:, cc, :])

            # ================= layers =================
            for li in range(L):
                wof = li * H
                b1c = bias_sb[:, 8 * li + 0:8 * li + 1]
                b2c = bias_sb[:, 8 * li + 1:8 * li + 2]
                gb1c = bias_sb[:, 8 * li + 2:8 * li + 3]
                gb2c = bias_sb[:, 8 * li + 3:8 * li + 4]
                catb1c = bias_sb[:, 8 * li + 4:8 * li + 5]
                bngc = bias_sb[:, 8 * li + 5:8 * li + 6]
                bnbc = bias_sb[:, 8 * li + 6:8 * li + 7]
                gbngc = bias_sb[:, 8 * li + 7:8 * li + 8]
                gbnbc = bias2_sb[:, 4 * li:4 * li + 1]

                # ---- A: intra aggregation + hlin ----
                GW = 4
                NT4 = GW * tpw
                for wg in range(NWIN // GW):
                    iw = sb.tile([P, NT4], I32, tag="aiw")
                    nc.sync.dma_start(iw[:], isrc[:, wg * NT4:(wg + 1) * NT4])
                    raw = sb.tile([P, NT4, H], BF16, tag="araw")
                    for j in range(NT4):
                        gather1(raw[:, j, :], h_d, iw[:, j:j + 1])
                    tohw = sb.tile([32, NT4, P], BF16, tag="atoh")
                    nc.sync.dma_start(
                        tohw[:], toh[:, wg * NT4 * P:(wg + 1) * NT4 * P])
                    dstc = sb.tile([P, NT4], F32, tag="adst")
                    nc.sync.dma_start(dstc[:],
                                      idst[:, wg * NT4:(wg + 1) * NT4])
                    ms = sb.tile([P, NT4, H], F32, tag="ams")
                    for j in range(NT4):
                        pb = ps.tile([P, H], F32, space="PSUM", tag="ps128b")
                        nc.tensor.matmul(out=pb[:], lhsT=tohw[:, j, :],
                                         rhs=bond_sb[:], start=True, stop=True)
                        nc.vector.tensor_tensor(out=ms[:, j, :],
                                                in0=raw[:, j, :], in1=pb[:],
                                                op=ALU.add)
                    msg = sb.tile([P, NT4, H], BF16, tag="amsg")
                    nc.scalar.activation(msg[:], ms[:], AF.Relu)
                    oneh = sb.tile([P, NT4, P], BF16, tag="aoneh")
                    nc.vector.tensor_tensor(
                        out=oneh[:],
                        in0=iota_sb[:, None, :].to_broadcast([P, NT4, P]),
                        in1=dstc[:, :, None].to_broadcast([P, NT4, P]),
                        op=ALU.is_equal)
                    hw = sb.tile([P, GW, H], BF16, tag="ahw")
                    nc.sync.dma_start(hw[:], h_pm[:, wg * GW:(wg + 1) * GW, :])
                    ht = sb.tile([P, GW, H], F32, tag="aht")
                    nc.vector.tensor_scalar(out=ht[:], in0=hw[:],
                                            scalar1=1.0 + eps_l[li],
                                            scalar2=None, op0=ALU.mult)
                    hl = sb.tile([P, GW, H], BF16, tag="ahl")
                    for k in range(GW):
                        pa = ps.tile([P, H], F32, space="PSUM", tag="ps128")
                        for j in range(tpw):
                            jj = k * tpw + j
                            nc.tensor.matmul(out=pa[:], lhsT=oneh[:, jj, :],
                                             rhs=msg[:, jj, :],
                                             start=(j == 0),
                                             stop=(j == tpw - 1))
                        nc.vector.tensor_tensor(out=hl[:, k, :], in0=pa[:],
                                                in1=ht[:, k, :], op=ALU.add)
                    for k in range(GW):
                        w = wg * GW + k
                        nc.sync.dma_start(hlin_d[w * P:(w + 1) * P, :],
                                          hl[:, k, :])

                # ---- A2: local MLP (feature-major) + BN stats ----
                sxc = sbw.tile([P, ROWS // 512], F32, tag="sxc")
                sqc = sbw.tile([P, ROWS // 512], F32, tag="sqc")
                w1 = sbw.tile([H, H], BF16, tag="w1")
                nc.sync.dma_start(w1[:], wl1[wof:wof + H, :])
                w2 = sbw.tile([H, H], BF16, tag="w2")
                nc.sync.dma_start(w2[:], wl2[wof:wof + H, :])
                for rt in range(ROWS // 512):
                    hT = sb.tile([H, 512], BF16, tag="m_hT")
                    nc.sync.dma_start_transpose(
                        hT[:], hlin_d[rt * 512:(rt + 1) * 512, :])
                    p1 = ps2.tile([H, 512], F32, space="PSUM", tag="ps512")
                    nc.tensor.matmul(out=p1[:], lhsT=w1[:], rhs=hT[:],
                                     start=True, stop=True)
                    mid = sb.tile([H, 512], BF16, tag="m_mid")
                    nc.scalar.activation(mid[:], p1[:], AF.Relu, bias=b1c)
                    p2 = ps2.tile([H, 512], F32, space="PSUM", tag="ps512")
                    nc.tensor.matmul(out=p2[:], lhsT=w2[:], rhs=mid[:],
                                     start=True, stop=True)
                    hr = sb.tile([H, 512], BF16, tag="m_hr")
                    nc.scalar.activation(hr[:], p2[:], AF.Relu, bias=b2c)
                    nc.vector.tensor_reduce(
                        out=sxc[:, rt:rt + 1], in_=hr[:],
                        axis=mybir.AxisListType.X, op=ALU.add)
                    sq_scr = sb.tile([H, 512], F32, tag="m_sq")
                    nc.scalar.activation(sq_scr[:], hr[:], AF.Square,
                                         accum_out=sqc[:, rt:rt + 1])
                    nc.sync.dma_start(hrT_d[:, rt * 512:(rt + 1) * 512], hr[:])

                # ---- BN local stats allreduce ----
                st = sb.tile([P, 2], F32, tag="st")
                nc.vector.tensor_reduce(out=st[:, 0:1], in_=sxc[:],
                                        axis=mybir.AxisListType.X, op=ALU.add)
                nc.vector.tensor_reduce(out=st[:, 1:2], in_=sqc[:],
                                        axis=mybir.AxisListType.X, op=ALU.add)
                nc.sync.dma_start(stat_in[:], st[:])
                nc.gpsimd.collective_compute(
                    "AllReduce", ALU.add, replica_groups=RG,
                    ins=[stat_in[:].opt()], outs=[stat_out[:].opt()])
                sg = sb.tile([P, 2], F32, tag="sg")
                nc.sync.dma_start(sg[:], stat_out[:])
                mu = sb.tile([P, 1], F32, tag="mu")
                nc.vector.tensor_scalar(out=mu[:], in0=sg[:, 0:1],
                                        scalar1=1.0 / SK, scalar2=None,
                                        op0=ALU.mult)
                var = sb.tile([P, 1], F32, tag="var")
                nc.vector.tensor_tensor(out=var[:], in0=mu[:], in1=mu[:],
                                        op=ALU.mult)
                v2 = sb.tile([P, 1], F32, tag="v2")
                nc.vector.tensor_scalar(out=v2[:], in0=sg[:, 1:2],
                                        scalar1=1.0 / SK, scalar2=None,
                                        op0=ALU.mult)
                nc.vector.tensor_tensor(out=var[:], in0=v2[:], in1=var[:],
                                        op=ALU.subtract)
                nc.vector.tensor_scalar(out=var[:], in0=var[:], scalar1=1e-5,
                                        scalar2=None, op0=ALU.add)
                sd = sb.tile([P, 1], F32, tag="sd")
                nc.scalar.activation(sd[:], var[:], AF.Sqrt)
                rs = sb.tile([P, 1], F32, tag="rs")
                nc.vector.reciprocal(rs[:], sd[:])
                a_bn = sb.tile([P, 1], F32, tag="a_bn")
                nc.vector.tensor_tensor(out=a_bn[:], in0=bngc, in1=rs[:],
                                        op=ALU.mult)
                nb = sb.tile([P, 1], F32, tag="nb")
                nc.vector.tensor_tensor(out=nb[:], in0=mu[:], in1=a_bn[:],
                                        op=ALU.mult)
                b_bn = sb.tile([P, 1], F32, tag="b_bn")
                nc.vector.tensor_tensor(out=b_bn[:], in0=bnbc, in1=nb[:],
                                        op=ALU.subtract)

                # ---- B1: h_node local + allgather ----
                for j in range(NODES // P):
                    ev = sb.tile([P, H], BF16, tag="b1e")
                    nc.sync.dma_start(
                        ev[:], h_d[j * 8192: (j + 1) * 8192: 64, :])
                    od = sb.tile([P, H], BF16, tag="b1o")
                    nc.sync.dma_start(
                        od[:], h_d[j * 8192 + 32: (j + 1) * 8192: 64, :])
                    s0 = sb.tile([P, H], F32, tag="b1s")
                    nc.vector.tensor_tensor(out=s0[:], in0=ev[:], in1=od[:],
                                            op=ALU.add)
                    hn = sb.tile([P, H], BF16, tag="b1h")
                    nc.vector.tensor_scalar(out=hn[:], in0=s0[:], scalar1=0.5,
                                            scalar2=None, op0=ALU.mult)
                    nc.sync.dma_start(hn_d[j * P:(j + 1) * P, :], hn[:])
                nc.gpsimd.collective_compute(
                    "AllGather", ALU.bypass, replica_groups=RG,
                    ins=[hn_d[:].opt()], outs=[hnfull_d[:].opt()])

                # ---- B2: global aggregation ----
                GWG = 4
                NTG = GWG * tpw_g
                for wg in range(NWIN_G // GWG):
                    giw = sb.tile([P, NTG], I32, tag="giw")
                    nc.sync.dma_start(giw[:],
                                      gsrc[:, wg * NTG:(wg + 1) * NTG])
                    raw = sb.tile([P, NTG, H], BF16, tag="graw")
                    for j in range(NTG):
                        gather1(raw[:, j, :], hnfull_d, giw[:, j:j + 1])
                    gtohw = sb.tile([32, NTG, P], BF16, tag="gtoh")
                    nc.sync.dma_start(
                        gtohw[:], gtoh[:, wg * NTG * P:(wg + 1) * NTG * P])
                    dstc = sb.tile([P, NTG], F32, tag="gdstc")
                    nc.sync.dma_start(dstc[:],
                                      gdst[:, wg * NTG:(wg + 1) * NTG])
                    ms = sb.tile([P, NTG, H], F32, tag="gms")
                    for j in range(NTG):
                        pb = ps.tile([P, H], F32, space="PSUM", tag="ps128b")
                        nc.tensor.matmul(out=pb[:], lhsT=gtohw[:, j, :],
                                         rhs=bond_sb[:], start=True, stop=True)
                        nc.vector.tensor_tensor(out=ms[:, j, :],
                                                in0=raw[:, j, :], in1=pb[:],
                                                op=ALU.add)
                    msg = sb.tile([P, NTG, H], BF16, tag="gmsg")
                    nc.scalar.activation(msg[:], ms[:], AF.Relu)
                    oneh = sb.tile([P, NTG, P], BF16, tag="goneh")
                    nc.vector.tensor_tensor(
                        out=oneh[:],
                        in0=iota_sb[:, None, :].to_broadcast([P, NTG, P]),
                        in1=dstc[:, :, None].to_broadcast([P, NTG, P]),
                        op=ALU.is_equal)
                    for k in range(GWG):
                        w = wg * GWG + k
                        pa = ps.tile([P, H], F32, space="PSUM", tag="ps128")
                        for j in range(tpw_g):
                            jj = k * tpw_g + j
                            nc.tensor.matmul(out=pa[:], lhsT=oneh[:, jj, :],
                                             rhs=msg[:, jj, :],
                                             start=(j == 0),
                                             stop=(j == tpw_g - 1))
                        ag = sb.tile([P, H], BF16, tag="gag")
                        nc.vector.tensor_copy(out=ag[:], in_=pa[:])
                        nc.sync.dma_start(aggN_d[w * P:(w + 1) * P, :], ag[:])
                nc.gpsimd.collective_compute(
                    "ReduceScatter", ALU.add, replica_groups=RG,
                    ins=[aggN_d[:].opt()], outs=[aggN_rs[:].opt()])

                # hlinN = (1+eps_g)*hn + aggN  (our 512 rows)
                for j in range(NODES // P):
                    hn = sb.tile([P, H], BF16, tag="b2h")
                    nc.sync.dma_start(hn[:], hn_d[j * P:(j + 1) * P, :])
                    ar = sb.tile([P, H], BF16, tag="b2a")
                    nc.sync.dma_start(ar[:], aggN_rs[j * P:(j + 1) * P, :])
                    t0 = sb.tile([P, H], F32, tag="b2t")
                    nc.vector.tensor_scalar(out=t0[:], in0=hn[:],
                                            scalar1=1.0 + eps_g[li],
                                            scalar2=None, op0=ALU.mult)
                    hl = sb.tile([P, H], BF16, tag="b2l")
                    nc.vector.tensor_tensor(out=hl[:], in0=t0[:], in1=ar[:],
                                            op=ALU.add)
                    nc.sync.dma_start(hlinN_d[j * P:(j + 1) * P, :], hl[:])

                # global MLP on 512 rows (feature-major, one tile)
                hT = sb.tile([H, NODES], BF16, tag="n_hT")
                nc.sync.dma_start_transpose(hT[:], hlinN_d[:, :])
                wg1 = sb.tile([H, H], BF16, tag="wg1")
                nc.sync.dma_start(wg1[:], gw1[wof:wof + H, :])
                p1 = ps2.tile([H, NODES], F32, space="PSUM", tag="ps512")
                nc.tensor.matmul(out=p1[:], lhsT=wg1[:], rhs=hT[:],
                                 start=True, stop=True)
                mid = sb.tile([H, NODES], BF16, tag="n_mid")
                nc.scalar.activation(mid[:], p1[:], AF.Relu, bias=gb1c)
                wg2 = sb.tile([H, H], BF16, tag="wg2")
                nc.sync.dma_start(wg2[:], gw2[wof:wof + H, :])
                p2 = ps2.tile([H, NODES], F32, space="PSUM", tag="ps512")
                nc.tensor.matmul(out=p2[:], lhsT=wg2[:], rhs=mid[:],
                                 start=True, stop=True)
                hcr = sb.tile([H, NODES], BF16, tag="n_hcr")
                nc.scalar.activation(hcr[:], p2[:], AF.Relu, bias=gb2c)
                # BN-global stats (local slice) + allreduce
                stg = sb.tile([P, 2], F32, tag="stg")
                nc.vector.tensor_reduce(out=stg[:, 0:1], in_=hcr[:],
                                        axis=mybir.AxisListType.X, op=ALU.add)
                sqg_scr = sb.tile([H, NODES], F32, tag="n_sq")
                nc.scalar.activation(sqg_scr[:], hcr[:], AF.Square,
                                     accum_out=stg[:, 1:2])
                nc.sync.dma_start(statg_in[:], stg[:])
                nc.gpsimd.collective_compute(
                    "AllReduce", ALU.add, replica_groups=RG,
                    ins=[statg_in[:].opt()], outs=[statg_out[:].opt()])
                sgo = sb.tile([P, 2], F32, tag="sgo")
                nc.sync.dma_start(sgo[:], statg_out[:])
                mug = sb.tile([P, 1], F32, tag="mug")
                nc.vector.tensor_scalar(out=mug[:], in0=sgo[:, 0:1],
                                        scalar1=1.0 / Nn, scalar2=None,
                                        op0=ALU.mult)
                varg = sb.tile([P, 1], F32, tag="varg")
                nc.vector.tensor_tensor(out=varg[:], in0=mug[:], in1=mug[:],
                                        op=ALU.mult)
                v2g = sb.tile([P, 1], F32, tag="v2g")
                nc.vector.tensor_scalar(out=v2g[:], in0=sgo[:, 1:2],
                                        scalar1=1.0 / Nn, scalar2=None,
                                        op0=ALU.mult)
                nc.vector.tensor_tensor(out=varg[:], in0=v2g[:], in1=varg[:],
                                        op=ALU.subtract)
                nc.vector.tensor_scalar(out=varg[:], in0=varg[:], scalar1=1e-5,
                                        scalar2=None, op0=ALU.add)
                sdg = sb.tile([P, 1], F32, tag="sdg")
                nc.scalar.activation(sdg[:], varg[:], AF.Sqrt)
                rsg = sb.tile([P, 1], F32, tag="rsg")
                nc.vector.reciprocal(rsg[:], sdg[:])
                ag_bn = sb.tile([P, 1], F32, tag="ag_bn")
                nc.vector.tensor_tensor(out=ag_bn[:], in0=gbngc, in1=rsg[:],
                                        op=ALU.mult)
                nbg = sb.tile([P, 1], F32, tag="nbg")
                nc.vector.tensor_tensor(out=nbg[:], in0=mug[:], in1=ag_bn[:],
                                        op=ALU.mult)
                bg_bn = sb.tile([P, 1], F32, tag="bg_bn")
                nc.vector.tensor_tensor(out=bg_bn[:], in0=gbnbc, in1=nbg[:],
                                        op=ALU.subtract)
                # h_node_new^T = hn^T + BN(hcr)
                hnT = sb.tile([H, NODES], BF16, tag="n_hnT")
                nc.sync.dma_start_transpose(hnT[:], hn_d[:, :])
                hcb = sb.tile([H, NODES], F32, tag="n_hcb")
                nc.vector.tensor_scalar(out=hcb[:], in0=hcr[:],
                                        scalar1=ag_bn[:], scalar2=bg_bn[:],
                                        op0=ALU.mult, op1=ALU.add)
                hnn = sb.tile([H, NODES], BF16, tag="n_hnn")
                nc.vector.tensor_tensor(out=hnn[:], in0=hcb[:], in1=hnT[:],
                                        op=ALU.add)
                # hb^T = bcast_w^T @ hnn^T
                wbc = sb.tile([H, H], BF16, tag="wbc")
                nc.sync.dma_start(wbc[:], bcw[wof:wof + H, :])
                p3 = ps2.tile([H, NODES], F32, space="PSUM", tag="ps512")
                nc.tensor.matmul(out=p3[:], lhsT=wbc[:], rhs=hnn[:],
                                 start=True, stop=True)
                hbT = sbw.tile([H, NODES], BF16, tag="hbT")
                nc.vector.tensor_copy(out=hbT[:], in_=p3[:])

                # ---- B3/B4: cat MLP + LN + residual ----
                wc1t = sbw.tile([H, H], BF16, tag="wc1t")
                nc.sync.dma_start(wc1t[:], cw1t[wof:wof + H, :])
                wc1b = sbw.tile([H, H], BF16, tag="wc1b")
                nc.sync.dma_start(wc1b[:], cw1b[wof:wof + H, :])
                wc2 = sbw.tile([H, H], BF16, tag="wc2")
                nc.sync.dma_start(wc2[:], cw2[wof:wof + H, :])
                cb2 = sbw.tile([P, H], F32, tag="cb2")
                nc.sync.dma_start(cb2[:], cb2rep[li * P:(li + 1) * P, :])
                lng = sbw.tile([P, H], F32, tag="lng")
                nc.sync.dma_start(lng[:], lngrep[li * P:(li + 1) * P, :])
                lnb = sbw.tile([P, H], F32, tag="lnb")
                nc.sync.dma_start(lnb[:], lnbrep[li * P:(li + 1) * P, :])
                for rt in range(ROWS // 512):
                    hrt = sb.tile([H, 512], BF16, tag="c_hrt")
                    nc.sync.dma_start(hrt[:],
                                      hrT_d[:, rt * 512:(rt + 1) * 512])
                    hbn = sb.tile([H, 512], BF16, tag="c_hbn")
                    nc.vector.tensor_scalar(out=hbn[:], in0=hrt[:],
                                            scalar1=a_bn[:], scalar2=b_bn[:],
                                            op0=ALU.mult, op1=ALU.add)
                    pc = ps2.tile([H, 512], F32, space="PSUM", tag="ps512")
                    nc.tensor.matmul(out=pc[:], lhsT=wc1t[:], rhs=hbn[:],
                                     start=True, stop=False)
                    hbe = hbT[:, rt * 8:(rt + 1) * 8, None]
                    nc.tensor.matmul(out=pc[:], lhsT=wc1b[:],
                                     rhs=hbe.to_broadcast([H, 8, 64]),
                                     start=False, stop=True)
                    mid2 = sb.tile([H, 512], BF16, tag="c_mid2")
                    nc.scalar.activation(mid2[:], pc[:], AF.Gelu, bias=catb1c)
                    pn = ps2.tile([P, 4, H], F32, space="PSUM", tag="ps512")
                    for j in range(4):
                        nc.tensor.matmul(out=pn[:, j, :],
                                         lhsT=mid2[:, j * P:(j + 1) * P],
                                         rhs=wc2[:], start=True, stop=True)
                    xn = sb.tile([P, 4, H], F32, tag="c_xn")
                    nc.vector.tensor_tensor(
                        out=xn[:], in0=pn[:],
                        in1=cb2[:, None, :].to_broadcast([P, 4, H]),
                        op=ALU.add)
                    mu4 = sb.tile([P, 4], F32, tag="c_mu4")
                    nc.vector.tensor_reduce(out=mu4[:], in_=xn[:],
                                            axis=mybir.AxisListType.X,
                                            op=ALU.add)
                    nc.vector.tensor_scalar(out=mu4[:], in0=mu4[:],
                                            scalar1=1.0 / H, scalar2=None,
                                            op0=ALU.mult)
                    sq4 = sb.tile([P, 4, H], F32, tag="c_sq4")
                    nc.vector.tensor_tensor(out=sq4[:], in0=xn[:], in1=xn[:],
                                            op=ALU.mult)
                    s24 = sb.tile([P, 4], F32, tag="c_s24")
                    nc.vector.tensor_reduce(out=s24[:], in_=sq4[:],
                                            axis=mybir.AxisListType.X,
                                            op=ALU.add)
                    nc.vector.tensor_scalar(out=s24[:], in0=s24[:],
                                            scalar1=1.0 / H, scalar2=None,
                                            op0=ALU.mult)
                    m2 = sb.tile([P, 4], F32, tag="c_m2")
                    nc.vector.tensor_tensor(out=m2[:], in0=mu4[:], in1=mu4[:],
                                            op=ALU.mult)
                    nc.vector.tensor_tensor(out=s24[:], in0=s24[:], in1=m2[:],
                                            op=ALU.subtract)
                    nc.vector.tensor_scalar(out=s24[:], in0=s24[:],
                                            scalar1=1e-5, scalar2=None,
                                            op0=ALU.add)
                    sd4 = sb.tile([P, 4], F32, tag="c_sd4")
                    nc.scalar.activation(sd4[:], s24[:], AF.Sqrt)
                    rs4 = sb.tile([P, 4], F32, tag="c_rs4")
                    nc.vector.reciprocal(rs4[:], sd4[:])
                    nc.vector.tensor_tensor(
                        out=xn[:], in0=xn[:],
                        in1=mu4[:, :, None].to_broadcast([P, 4, H]),
                        op=ALU.subtract)
                    nc.vector.tensor_tensor(
                        out=xn[:], in0=xn[:],
                        in1=rs4[:, :, None].to_broadcast([P, 4, H]),
                        op=ALU.mult)
                    nc.vector.tensor_tensor(
                        out=xn[:], in0=xn[:],
                        in1=lng[:, None, :].to_broadcast([P, 4, H]),
                        op=ALU.mult)
                    nc.vector.tensor_tensor(
                        out=xn[:], in0=xn[:],
                        in1=lnb[:, None, :].to_broadcast([P, 4, H]),
                        op=ALU.add)
                    hin = sb.tile([P, 4, H], BF16, tag="c_hin")
                    nc.sync.dma_start(hin[:],
                                      h_pm[:, rt * 4:(rt + 1) * 4, :])
                    nc.vector.tensor_tensor(out=xn[:], in0=xn[:], in1=hin[:],
                                            op=ALU.add)
                    hout = sb.tile([P, 4, H], BF16, tag="c_hout")
                    vsl = validf_sb[:, rt * 4:(rt + 1) * 4, None]
                    nc.vector.tensor_tensor(
                        out=hout[:], in0=xn[:],
                        in1=vsl.to_broadcast([P, 4, H]), op=ALU.mult)
                    nc.sync.dma_start(h_pm[:, rt * 4:(rt + 1) * 4, :],
                                      hout[:])
                    for j in range(4):
                        nc.sync.dma_start(
                            h_d[rt * 512 + j * P: rt * 512 + (j + 1) * P, :],
                            hout[:, j, :])

            # ================= pooling =================
            pp = pspool.tile([NG, H], F32, space="PSUM", tag="poolps")
            for rt in range(NWIN):
                htile = sb.tile([P, H], BF16, tag="p_h")
                nc.sync.dma_start(htile[:], h_pm[:, rt, :])
                wp = sb.tile([P, NG], BF16, tag="p_w")
                nc.sync.dma_start(wp[:], wpool[:, rt * NG:(rt + 1) * NG])
                nc.tensor.matmul(out=pp[:], lhsT=wp[:], rhs=htile[:],
                                 start=(rt == 0), stop=(rt == NWIN - 1))
            po = sb.tile([NG, H], F32, tag="p_o")
            nc.vector.tensor_copy(out=po[:], in_=pp[:])
            nc.sync.dma_start(pool_in[:], po[:])
            nc.gpsimd.collective_compute(
                "AllReduce", ALU.add, replica_groups=RG,
                ins=[pool_in[:].opt()], outs=[pool_out[:].opt()])
            fo = sb.tile([NG, H], F32, tag="p_f")
            nc.sync.dma_start(fo[:], pool_out[:])
            nc.sync.dma_start(out_ext[:], fo[:])

    nc.finalize()
    return nc


# ----------------------------------------------------------------------------
# kernel entry
# ----------------------------------------------------------------------------

def kernel(**inputs):
    np64 = lambda x: np.asarray(x)
    atom_emb = np.asarray(inputs["atom_emb"], np.float32)
    bond_emb = np.asarray(inputs["bond_emb"], np.float32)
    rwse_w = np.asarray(inputs["rwse_w"], np.float32)
    rwse_b = np.asarray(inputs["rwse_b"], np.float32)
    rwse = np.asarray(inputs["rwse"], np.float32)
    l_eps = np.asarray(inputs["l_eps"], np.float32)
    l_w1 = np.asarray(inputs["l_w1"], np.float32)
    l_b1 = np.asarray(inputs["l_b1"], np.float32)
    l_w2 = np.asarray(inputs["l_w2"], np.float32)
    l_b2 = np.asarray(inputs["l_b2"], np.float32)
    l_bng = np.asarray(inputs["l_bng"], np.float32)
    l_bnb = np.asarray(inputs["l_bnb"], np.float32)
    g_eps = np.asarray(inputs["g_eps"], np.float32)
    g_w1 = np.asarray(inputs["g_w1"], np.float32)
    g_b1 = np.asarray(inputs["g_b1"], np.float32)
    g_w2 = np.asarray(inputs["g_w2"], np.float32)
    g_b2 = np.asarray(inputs["g_b2"], np.float32)
    g_bng = np.asarray(inputs["g_bng"], np.float32)
    g_bnb = np.asarray(inputs["g_bnb"], np.float32)
    bcast_w = np.asarray(inputs["bcast_w"], np.float32)
    cat_w1 = np.asarray(inputs["cat_w1"], np.float32)
    cat_b1 = np.asarray(inputs["cat_b1"], np.float32)
    cat_w2 = np.asarray(inputs["cat_w2"], np.float32)
    cat_b2 = np.asarray(inputs["cat_b2"], np.float32)
    ln_g = np.asarray(inputs["ln_g"], np.float32)
    ln_b = np.asarray(inputs["ln_b"], np.float32)
    x_ids = np.asarray(inputs["x_ids"], np.int64)
    intra_ei = np.asarray(inputs["intra_ei"], np.int64)
    intra_ea_ids = np.asarray(inputs["intra_ea_ids"], np.int64)
    global_ei = np.asarray(inputs["global_ei"], np.int64)
    global_ea_ids = np.asarray(inputs["global_ea_ids"], np.int64)
    node_ids = np.asarray(inputs["node_ids"], np.int64)
    valid = np.asarray(inputs["valid"], np.int64)
    batch = np.asarray(inputs["batch"], np.int64)

    bond_ext = np.zeros((32, H), np.float32)
    bond_ext[:16] = bond_emb
    bond_ext[16] = -1e4

    # ---- per-core edge packing (intra) ----
    esrc, edst = intra_ei[0], intra_ei[1]
    esub = edst // K_SUB
    ecore = esub // SUBS
    packed = []
    for c in range(NCORES):
        m = ecore == c
        s = esrc[m] - c * ROWS
        d = edst[m] - c * ROWS
        t = intra_ea_ids[m]
        packed.append(pack_edges(s, d, t, ROWS, NWIN))
    tpw = 1
    for (ts, tt, tr, tw) in packed:
        cnt = np.bincount(tw, minlength=NWIN)
        tpw = max(tpw, int(cnt.max()))
    intra = [layout_windows(*pk, NWIN, tpw) for pk in packed]

    # ---- per-core global edge packing ----
    gsrc_, gdst_ = global_ei[0], global_ei[1]
    Eg = len(gsrc_)
    epc = Eg // NCORES
    gpacked = []
    for c in range(NCORES):
        sl = slice(c * epc, (c + 1) * epc)
        gpacked.append(pack_edges(gsrc_[sl], gdst_[sl],
                                  global_ea_ids[sl], Nn, NWIN_G))
    tpw_g = 1
    for (ts, tt, tr, tw) in gpacked:
        cnt = np.bincount(tw, minlength=NWIN_G)
        tpw_g = max(tpw_g, int(cnt.max()))
    gintra = [layout_windows(*pk, NWIN_G, tpw_g) for pk in gpacked]

    # ---- pooling weights per core ----
    valid_f = valid.astype(np.float32)
    cnt_s = valid_f.reshape(S_TOT, K_SUB).sum(1)
    wrow = 1.0 / (2.0 * np.maximum(cnt_s, 1.0))       # per subgraph
    node_of_sub = np.arange(S_TOT) // M_SUB
    graph_of_sub = batch[node_of_sub]                  # [S_TOT]

    nc = build_program(tpw, tpw_g, [float(x) for x in l_eps],
                       [float(x) for x in g_eps])

    in_maps = []
    for c in range(NCORES):
        r0 = c * ROWS
        d = {}
        d["atom"] = atom_emb.astype(BF)
        d["bond"] = bond_ext.astype(BF)
        d["rwse_lt"] = np.ascontiguousarray(rwse.T).astype(BF)
        d["rwse_w"] = rwse_w.astype(BF)
        d["rwse_brep"] = np.broadcast_to(rwse_b, (P, H)).astype(np.float32).copy()
        d["wl1"] = l_w1.reshape(L * H, H).astype(BF)
        d["wl2"] = l_w2.reshape(L * H, H).astype(BF)
        d["gw1"] = g_w1.reshape(L * H, H).astype(BF)
        d["gw2"] = g_w2.reshape(L * H, H).astype(BF)
        d["bcw"] = bcast_w.reshape(L * H, H).astype(BF)
        d["cw1t"] = cat_w1[:, :H, :].reshape(L * H, H).astype(BF)
        d["cw1b"] = cat_w1[:, H:, :].reshape(L * H, H).astype(BF)
        d["cw2"] = cat_w2.reshape(L * H, H).astype(BF)
        bias_cols = np.zeros((P, 8 * L), np.float32)
        bias2_cols = np.zeros((P, 4 * L), np.float32)
        for li in range(L):
            bias_cols[:, 8 * li + 0] = l_b1[li]
            bias_cols[:, 8 * li + 1] = l_b2[li]
            bias_cols[:, 8 * li + 2] = g_b1[li]
            bias_cols[:, 8 * li + 3] = g_b2[li]
            bias_cols[:, 8 * li + 4] = cat_b1[li]
            bias_cols[:, 8 * li + 5] = l_bng[li]
            bias_cols[:, 8 * li + 6] = l_bnb[li]
            bias_cols[:, 8 * li + 7] = g_bng[li]
            bias2_cols[:, 4 * li] = g_bnb[li]
        d["bias_cols"] = bias_cols
        d["bias2_cols"] = bias2_cols
        d["cb2rep"] = np.repeat(cat_b2[:, None, :], P, 1).reshape(L * P, H).astype(np.float32)
        d["lngrep"] = np.repeat(ln_g[:, None, :], P, 1).reshape(L * P, H).astype(np.float32)
        d["lnbrep"] = np.repeat(ln_b[:, None, :], P, 1).reshape(L * P, H).astype(np.float32)
        d["iota_rep"] = np.broadcast_to(np.arange(P, dtype=np.float32), (P, P)).copy()
        vloc = valid_f[r0:r0 + ROWS]
        d["validf"] = np.ascontiguousarray(vloc.reshape(NWIN, P).T)
        wp = np.zeros((ROWS, NG), np.float32)
        for s in range(SUBS):
            gs = c * SUBS + s
            wp[s * K_SUB:(s + 1) * K_SUB, graph_of_sub[gs]] = wrow[gs]
        d["wpool"] = np.ascontiguousarray(
            wp.reshape(NWIN, P, NG).transpose(1, 0, 2).reshape(P, NWIN * NG)).astype(BF)
        d["x32"] = np.ascontiguousarray(
            x_ids[r0:r0 + ROWS].reshape(NWIN, P).T).astype(np.int32)
        d["n32"] = np.ascontiguousarray(
            node_ids[r0:r0 + ROWS].reshape(NWIN, P).T).astype(np.int32)
        srcw, typw, relw = intra[c]
        d["isrc"] = np.ascontiguousarray(
            srcw.reshape(NWIN * tpw, P).T).astype(np.int32)
        tohv = np.zeros((32, NWIN * tpw * P), BF)
        tohv[typw.reshape(-1).astype(np.int64),
             np.arange(NWIN * tpw * P)] = 1.0
        d["toh"] = tohv
        d["idst"] = np.ascontiguousarray(
            relw.reshape(NWIN * tpw, P).T).astype(np.float32)
        gsw, gtw, grw = gintra[c]
        d["gsrc"] = np.ascontiguousarray(
            gsw.reshape(NWIN_G * tpw_g, P).T).astype(np.int32)
        gtohv = np.zeros((32, NWIN_G * tpw_g * P), BF)
        gtohv[gtw.reshape(-1).astype(np.int64),
              np.arange(NWIN_G * tpw_g * P)] = 1.0
        d["gtoh"] = gtohv
        d["gdst"] = np.ascontiguousarray(
            grw.reshape(NWIN_G * tpw_g, P).T).astype(np.float32)
        in_maps.append(d)

    res = run_bass_kernel_spmd(nc, in_maps, list(range(NCORES)),
                               **_extra_run_kwargs())
    out = res.results[0]["out"]
    kernel.last_exec_ns = res.exec_time_ns
    return np.asarray(out, np.float32)


def _extra_run_kwargs():
    kw = {}
    if os.environ.get("BASS_KERNEL_TRACE"):
        kw["trace"] = True
    return kw


kernel.last_exec_ns = None



# revision 20
# speedup vs baseline: 1.8014x; 1.8014x over previous
"""Trainium2 distributed Bass kernel for the hierarchical GNN encoder.

Strategy (8 NeuronCores, SPMD):
  - Shard the S=8192 subgraphs contiguously: 1024 subgraphs (=32768 flat rows,
    512 original nodes) per core.  Intra edges are subgraph-local so each
    core's intra edges are fully local, and src/dst of every edge live in the
    same 128-row window (4 subgraphs).
  - NO indirect row gathers for the intra graph: messages are computed as
    PE matmuls against host-precomputed one-hot selection tables:
      msg  = selT^T @ h_window + toh^T @ bond      (relu on Scalar engine)
      agg += dstoh^T @ msg                          (PSUM accumulation)
    Edges are dst-sorted per window and cut into 128-edge tiles with no
    run/packing constraints (duplicate dst rows accumulate in PSUM).
  - h0 init is also matmul-only: atom one-hot table and a host-gathered
    rwse[node_ids] table (with a ones row folding in the bias).
  - Global graph: edges are owned by the dst-window owner (no ReduceScatter);
    h_node[src] rows come from ONE batched SWDGE dma_gather out of the
    all-gathered h_node table per layer.
  - MLPs run on PE with weights stationary; BatchNorm batch stats are
    all-reduced ([H,2] per norm); the final pooled [64,H] output is
    all-reduced.
"""

import math
import os
import sys

sys.path.insert(0, "/opt/trn_rl_repo")

import numpy as np
import ml_dtypes

from concourse import bacc, bass, mybir, tile
from concourse.bass_utils import run_bass_kernel_spmd

P = 128
H = 128
L = 4
NCORES = 8
NG = 64
Nn = 4096
M_SUB = 2          # subgraphs per node
K_SUB = 32         # nodes per subgraph
S_TOT = Nn * M_SUB
SK = S_TOT * K_SUB
ROWS = SK // NCORES          # 32768 flat rows per core
SUBS = S_TOT // NCORES       # 1024 subgraphs per core
NODES = Nn // NCORES         # 512 nodes per core
NWIN = ROWS // P             # 256 agg windows per core
NWIN_G = NODES // P          # 4 global dst windows owned per core
F32 = mybir.dt.float32
BF16 = mybir.dt.bfloat16
I16 = mybir.dt.int16
I32 = mybir.dt.int32
AF = mybir.ActivationFunctionType
ALU = mybir.AluOpType
BF = ml_dtypes.bfloat16
GW = 4             # windows per intra group


# ----------------------------------------------------------------------------
# Host-side edge table construction
# ----------------------------------------------------------------------------

def tile_counts(dst_sorted_win, n_win, n_cores_lists):
    """Per-window tile counts (max across cores)."""
    nt = np.ones(n_win, np.int64)
    for counts in n_cores_lists:
        nt = np.maximum(nt, (counts + P - 1) // P)
    return nt


def build_intra_tables(s, d, t, nt, off):
    """One-hot tables for a core's intra edges (dst-sorted within window).

    s, d: row ids relative to core (0..ROWS); t: bond type.
    nt: per-window tile count (shared across cores); off: prefix offsets.
    Returns selT [128, TT*128], toh [32, TT*128], dstoh [128, TT*128] (bf16).
    """
    TT = int(off[-1])
    w = d // P
    order = np.argsort(w, kind="stable")
    s, d, t, w = s[order], d[order], t[order], w[order]
    # position within window
    cnt = np.bincount(w, minlength=len(nt))
    starts = np.concatenate([[0], np.cumsum(cnt)[:-1]])
    pos = np.arange(len(d)) - starts[w]
    col = (off[w] + pos // P) * P + pos % P
    selT = np.zeros((P, TT * P), BF)
    toh = np.zeros((32, TT * P), BF)
    dstoh = np.zeros((P, TT * P), BF)
    selT[s - w * P, col] = 1.0
    toh[t, col] = 1.0
    dstoh[pos % P, (off[w] + pos // P) * P + (d - w * P)] = 1.0
    return selT, toh, dstoh


def build_global_tables(s, d, t, ntg, offg, win_base):
    """Tables for a core's owned global edges (dst in core's 4 windows).

    Returns gsrc_flat [TTG*128] int64 (pad 0), gtoh [32, TTG*128],
    gdstoh [128, TTG*128].
    """
    TTG = int(offg[-1])
    w = (d - win_base) // P
    order = np.argsort(w, kind="stable")
    s, d, t, w = s[order], d[order], t[order], w[order]
    cnt = np.bincount(w, minlength=len(ntg))
    starts = np.concatenate([[0], np.cumsum(cnt)[:-1]])
    pos = np.arange(len(d)) - starts[w]
    col = (offg[w] + pos // P) * P + pos % P
    gsrc = np.zeros(TTG * P, np.int64)
    gtoh = np.zeros((32, TTG * P), BF)
    gdstoh = np.zeros((P, TTG * P), BF)
    gsrc[col] = s
    gtoh[t, col] = 1.0
    gdstoh[pos % P, (offg[w] + pos // P) * P + ((d - win_base) - w * P)] = 1.0
    return gsrc, gtoh, gdstoh


def wrap16_rep(idx):
    """[n] int -> [128, n//16] int16: wrapped 16-partition layout replicated
    8x across the 128 partitions (dma_gather index format)."""
    n = len(idx)
    assert n % 16 == 0
    w = np.ascontiguousarray(idx.reshape(n // 16, 16).T.astype(np.int16))
    return np.ascontiguousarray(np.tile(w, (8, 1)))


# ----------------------------------------------------------------------------
# Device program
# ----------------------------------------------------------------------------

def build_program(NT, NTG, eps_l, eps_g):
    NT = [int(x) for x in NT]
    NTG = [int(x) for x in NTG]
    OFF = np.concatenate([[0], np.cumsum(NT)]).astype(np.int64)
    OFFG = np.concatenate([[0], np.cumsum(NTG)]).astype(np.int64)
    TT = int(OFF[-1])
    TTG = int(OFFG[-1])

    nc = bacc.Bacc(None, target_bir_lowering=False, debug=True)

    def inp(name, shape, dtype):
        return nc.declare_dram_parameter(name, list(shape), dtype, isOutput=False)

    # weights / tables
    atom = inp("atom", [P, H], BF16)
    bond = inp("bond", [32, H], BF16)           # 16 real rows, rest zero
    rww = inp("rww", [32, H], BF16)             # rwse_w rows 0-15, b at row 16
    xoh = inp("xoh", [P, ROWS], BF16)           # atom one-hot (valid-masked)
    rw17 = inp("rw17", [32, ROWS], BF16)        # rwse[node_ids]^T + ones row 16
    tabs = inp("tabs", [P, TT * 2 * P], BF16)   # [selT | dstoh] per tile
    tohi = inp("tohi", [32, TT * P], BF16)
    gtoh = inp("gtoh", [32, TTG * P], BF16)
    gdstoh = inp("gdstoh", [P, TTG * P], BF16)
    gidx = inp("gidx", [P, TTG], I32)
    wl1 = inp("wl1", [L * H, H], BF16)
    wl2 = inp("wl2", [L * H, H], BF16)
    gw1 = inp("gw1", [L * H, H], BF16)
    gw2 = inp("gw2", [L * H, H], BF16)
    bcw = inp("bcw", [L * H, H], BF16)
    cw1t = inp("cw1t", [L * H, H], BF16)        # cat_w1 top half
    cw1b = inp("cw1b", [L * H, H], BF16)        # cat_w1 bottom half
    cw2 = inp("cw2", [L * H, H], BF16)
    bias_cols = inp("bias_cols", [P, 8 * L], F32)
    bias2_cols = inp("bias2_cols", [P, 4 * L], F32)
    cb2rep = inp("cb2rep", [L * P, H], F32)     # cat_b2 replicated per layer
    lngrep = inp("lngrep", [L * P, H], F32)
    lnbrep = inp("lnbrep", [L * P, H], F32)
    validf = inp("validf", [P, NWIN], F32)
    wpool = inp("wpool", [P, NWIN * NG], BF16)

    out_ext = nc.declare_dram_parameter("out", [NG, H], F32, isOutput=True)

    # internal DRAM
    h_pm = nc.dram_tensor("h_pm", [P, NWIN, H], BF16)
    hlin_d = nc.dram_tensor("hlin_d", [ROWS, H], BF16)
    hrT_d = nc.dram_tensor("hrT_d", [H, ROWS], BF16)
    hn_d = nc.dram_tensor("hn_d", [NODES, H], BF16)
    hnfull_d = nc.dram_tensor("hnfull_d", [Nn, H], BF16)
    hlinN_d = nc.dram_tensor("hlinN_d", [NODES, H], BF16)
    stat_in = nc.dram_tensor("stat_in", [P, 2], F32)
    stat_out = nc.dram_tensor("stat_out", [P, 2], F32)
    statg_in = nc.dram_tensor("statg_in", [P, 2], F32)
    statg_out = nc.dram_tensor("statg_out", [P, 2], F32)
    pool_in = nc.dram_tensor("pool_in", [NG, H], F32)
    pool_out = nc.dram_tensor("pool_out", [NG, H], F32)

    RG = [list(range(NCORES))]

    with tile.TileContext(nc) as tc:
        with (
            tc.tile_pool(name="const", bufs=1) as cpool,
            tc.tile_pool(name="sb", bufs=2) as sb,
            tc.tile_pool(name="sbw", bufs=2) as sbw,
            tc.tile_pool(name="gp", bufs=2) as gp,
            tc.tile_pool(name="ps", bufs=2, space="PSUM") as ps,
            tc.tile_pool(name="psagg", bufs=2, space="PSUM") as psagg,
            tc.tile_pool(name="ps2", bufs=2, space="PSUM") as ps2,
            tc.tile_pool(name="pspool", bufs=1, space="PSUM") as pspool,
        ):
            # ---- constants resident in SBUF ----
            bias_sb = cpool.tile([P, 8 * L], F32)
            nc.sync.dma_start(bias_sb[:], bias_cols[:])
            bias2_sb = cpool.tile([P, 4 * L], F32)
            nc.sync.dma_start(bias2_sb[:], bias2_cols[:])
            validf_sb = cpool.tile([P, NWIN], F32)
            nc.sync.dma_start(validf_sb[:], validf[:])
            bond_sb = cpool.tile([32, H], BF16)
            nc.sync.dma_start(bond_sb[:], bond[:])
            atom_sb = cpool.tile([P, H], BF16)
            nc.sync.dma_start(atom_sb[:], atom[:])
            rww_sb = cpool.tile([32, H], BF16)
            nc.sync.dma_start(rww_sb[:], rww[:])
            gidx_sb = cpool.tile([P, TTG], I32)
            nc.sync.dma_start(gidx_sb[:], gidx[:])

            # ================= init: h0 = atomsel + relu(rwsesel@W + b) =====
            for g in range(NWIN // GW):
                c0, c1 = g * GW * P, (g + 1) * GW * P
                xo = sb.tile([P, GW, P], BF16, tag="h0xo")
                nc.sync.dma_start(xo[:], xoh[:, c0:c1])
                rw = sb.tile([32, GW, P], BF16, tag="h0rw")
                nc.sync.dma_start(rw[:], rw17[:, c0:c1])
                psa = ps.tile([P, GW, H], F32, space="PSUM", tag="psmsg")
                psr = psagg.tile([P, GW, H], F32, space="PSUM", tag="psagg")
                for j in range(GW):
                    nc.tensor.matmul(out=psa[:, j, :], lhsT=xo[:, j, :],
                                     rhs=atom_sb[:], start=True, stop=True)
                    nc.tensor.matmul(out=psr[:, j, :], lhsT=rw[:, j, :],
                                     rhs=rww_sb[:], start=True, stop=True)
                rr = sb.tile([P, GW, H], BF16, tag="h0rr")
                nc.scalar.activation(rr[:], psr[:], AF.Relu)
                h0 = sb.tile([P, GW, H], BF16, tag="h0h")
                nc.vector.tensor_tensor(out=h0[:], in0=psa[:], in1=rr[:],
                                        op=ALU.add)
                nc.sync.dma_start(h_pm[:, g * GW:(g + 1) * GW, :], h0[:])

            # ================= layers =================
            for li in range(L):
                wof = li * H
                b1c = bias_sb[:, 8 * li + 0:8 * li + 1]
                b2c = bias_sb[:, 8 * li + 1:8 * li + 2]
                gb1c = bias_sb[:, 8 * li + 2:8 * li + 3]
                gb2c = bias_sb[:, 8 * li + 3:8 * li + 4]
                catb1c = bias_sb[:, 8 * li + 4:8 * li + 5]
                bngc = bias_sb[:, 8 * li + 5:8 * li + 6]
                bnbc = bias_sb[:, 8 * li + 6:8 * li + 7]
                gbngc = bias_sb[:, 8 * li + 7:8 * li + 8]
                gbnbc = bias2_sb[:, 4 * li:4 * li + 1]

                # ---- A: intra aggregation + hlin ----
                for g in range(NWIN // GW):
                    nt_k = NT[g * GW:(g + 1) * GW]
                    ntg = sum(nt_k)
                    o0 = int(OFF[g * GW])
                    tabc = sb.tile([P, ntg, 2, P], BF16, tag="a_tab", bufs=3)
                    nc.sync.dma_start(tabc[:],
                                      tabs[:, o0 * 2 * P:(o0 + ntg) * 2 * P])
                    tohc = sb.tile([32, ntg * P], BF16, tag="a_toh", bufs=3)
                    nc.sync.dma_start(tohc[:],
                                      tohi[:, o0 * P:(o0 + ntg) * P])
                    hw = sb.tile([P, GW, H], BF16, tag="a_hw", bufs=3)
                    nc.sync.dma_start(hw[:], h_pm[:, g * GW:(g + 1) * GW, :])
                    agg = psagg.tile([P, GW, H], F32, space="PSUM", tag="psagg")
                    tlist = [(k, j) for k in range(GW) for j in range(nt_k[k])]
                    for b0 in range(0, ntg, 4):
                        bl = tlist[b0:b0 + 4]
                        mps = ps.tile([P, GW, H], F32, space="PSUM", tag="psmsg")
                        for i, (k, j) in enumerate(bl):
                            t = b0 + i
                            nc.tensor.matmul(out=mps[:, i, :],
                                             lhsT=tabc[:, t, 0, :],
                                             rhs=hw[:, k, :],
                                             start=True, stop=False)
                            nc.tensor.matmul(out=mps[:, i, :],
                                             lhsT=tohc[:, t * P:(t + 1) * P],
                                             rhs=bond_sb[:],
                                             start=False, stop=True)
                        msg = sb.tile([P, GW, H], BF16, tag="a_msg")
                        nc.scalar.activation(msg[:, :len(bl), :],
                                             mps[:, :len(bl), :], AF.Relu)
                        for i, (k, j) in enumerate(bl):
                            t = b0 + i
                            nc.tensor.matmul(out=agg[:, k, :],
                                             lhsT=tabc[:, t, 1, :],
                                             rhs=msg[:, i, :],
                                             start=(j == 0),
                                             stop=(j == nt_k[k] - 1))
                    htf = sb.tile([P, GW, H], F32, tag="a_htf")
                    nc.vector.tensor_scalar(out=htf[:], in0=hw[:],
                                            scalar1=1.0 + eps_l[li],
                                            scalar2=None, op0=ALU.mult)
                    hl = sb.tile([P, GW, H], BF16, tag="a_hl")
                    nc.vector.tensor_tensor(out=hl[:], in0=agg[:], in1=htf[:],
                                            op=ALU.add)
                    nc.sync.dma_start(
                        hlin_d[g * GW * P:(g + 1) * GW * P, :].rearrange(
                            "(a b) h -> b a h", a=GW), hl[:])

                # ---- A2: local MLP (feature-major) + BN stats ----
                sxc = sbw.tile([P, ROWS // 512], F32, tag="sxc")
                sqc = sbw.tile([P, ROWS // 512], F32, tag="sqc")
                w1 = sbw.tile([H, H], BF16, tag="w1")
                nc.sync.dma_start(w1[:], wl1[wof:wof + H, :])
                w2 = sbw.tile([H, H], BF16, tag="w2")
                nc.sync.dma_start(w2[:], wl2[wof:wof + H, :])
                for rt in range(ROWS // 512):
                    hT = sb.tile([H, 512], BF16, tag="m_hT")
                    nc.sync.dma_start_transpose(
                        hT[:], hlin_d[rt * 512:(rt + 1) * 512, :])
                    p1 = ps2.tile([H, 512], F32, space="PSUM", tag="ps512")
                    nc.tensor.matmul(out=p1[:], lhsT=w1[:], rhs=hT[:],
                                     start=True, stop=True)
                    mid = sb.tile([H, 512], BF16, tag="m_mid")
                    nc.scalar.activation(mid[:], p1[:], AF.Relu, bias=b1c)
                    p2 = ps2.tile([H, 512], F32, space="PSUM", tag="ps512")
                    nc.tensor.matmul(out=p2[:], lhsT=w2[:], rhs=mid[:],
                                     start=True, stop=True)
                    hr = sb.tile([H, 512], BF16, tag="m_hr")
                    nc.scalar.activation(hr[:], p2[:], AF.Relu, bias=b2c)
                    nc.vector.tensor_reduce(
                        out=sxc[:, rt:rt + 1], in_=hr[:],
                        axis=mybir.AxisListType.X, op=ALU.add)
                    sq_scr = sb.tile([H, 512], F32, tag="m_sq")
                    nc.scalar.activation(sq_scr[:], hr[:], AF.Square,
                                         accum_out=sqc[:, rt:rt + 1])
                    nc.sync.dma_start(hrT_d[:, rt * 512:(rt + 1) * 512], hr[:])

                # ---- BN local stats allreduce ----
                st = sb.tile([P, 2], F32, tag="st")
                nc.vector.tensor_reduce(out=st[:, 0:1], in_=sxc[:],
                                        axis=mybir.AxisListType.X, op=ALU.add)
                nc.vector.tensor_reduce(out=st[:, 1:2], in_=sqc[:],
                                        axis=mybir.AxisListType.X, op=ALU.add)
                nc.sync.dma_start(stat_in[:], st[:])
                nc.gpsimd.collective_compute(
                    "AllReduce", ALU.add, replica_groups=RG,
                    ins=[stat_in[:].opt()], outs=[stat_out[:].opt()])
                sg = sb.tile([P, 2], F32, tag="sg")
                nc.sync.dma_start(sg[:], stat_out[:])
                mu = sb.tile([P, 1], F32, tag="mu")
                nc.vector.tensor_scalar(out=mu[:], in0=sg[:, 0:1],
                                        scalar1=1.0 / SK, scalar2=None,
                                        op0=ALU.mult)
                var = sb.tile([P, 1], F32, tag="var")
                nc.vector.tensor_tensor(out=var[:], in0=mu[:], in1=mu[:],
                                        op=ALU.mult)
                v2 = sb.tile([P, 1], F32, tag="v2")
                nc.vector.tensor_scalar(out=v2[:], in0=sg[:, 1:2],
                                        scalar1=1.0 / SK, scalar2=None,
                                        op0=ALU.mult)
                nc.vector.tensor_tensor(out=var[:], in0=v2[:], in1=var[:],
                                        op=ALU.subtract)
                nc.vector.tensor_scalar(out=var[:], in0=var[:], scalar1=1e-5,
                                        scalar2=None, op0=ALU.add)
                sd = sb.tile([P, 1], F32, tag="sd")
                nc.scalar.activation(sd[:], var[:], AF.Sqrt)
                rs = sb.tile([P, 1], F32, tag="rs")
                nc.vector.reciprocal(rs[:], sd[:])
                a_bn = sb.tile([P, 1], F32, tag="a_bn")
                nc.vector.tensor_tensor(out=a_bn[:], in0=bngc, in1=rs[:],
                                        op=ALU.mult)
                nb = sb.tile([P, 1], F32, tag="nb")
                nc.vector.tensor_tensor(out=nb[:], in0=mu[:], in1=a_bn[:],
                                        op=ALU.mult)
                b_bn = sb.tile([P, 1], F32, tag="b_bn")
                nc.vector.tensor_tensor(out=b_bn[:], in0=bnbc, in1=nb[:],
                                        op=ALU.subtract)

                # ---- B1: h_node from subgraph roots (h_pm partitions
                # 0/32/64/96 hold the roots of subgraphs w*4+0..3) ----
                rt0 = sb.tile([P, 2, H], BF16, tag="b1p0")
                nc.sync.dma_start(
                    rt0[:], h_pm[0, :, :].rearrange("(a b) h -> b a h", a=2))
                rt1 = sb.tile([P, 2, H], BF16, tag="b1p1")
                nc.sync.dma_start(
                    rt1[:], h_pm[32, :, :].rearrange("(a b) h -> b a h", a=2))
                rt2 = sb.tile([P, 2, H], BF16, tag="b1p2")
                nc.sync.dma_start(
                    rt2[:], h_pm[64, :, :].rearrange("(a b) h -> b a h", a=2))
                rt3 = sb.tile([P, 2, H], BF16, tag="b1p3")
                nc.sync.dma_start(
                    rt3[:], h_pm[96, :, :].rearrange("(a b) h -> b a h", a=2))
                se = sb.tile([P, 2, H], F32, tag="b1se")
                nc.vector.tensor_tensor(out=se[:], in0=rt0[:], in1=rt1[:],
                                        op=ALU.add)
                seh = sb.tile([P, 2, H], BF16, tag="b1seh")
                nc.vector.tensor_scalar(out=seh[:], in0=se[:], scalar1=0.5,
                                        scalar2=None, op0=ALU.mult)
                so = sb.tile([P, 2, H], F32, tag="b1so")
                nc.vector.tensor_tensor(out=so[:], in0=rt2[:], in1=rt3[:],
                                        op=ALU.add)
                soh = sb.tile([P, 2, H], BF16, tag="b1soh")
                nc.vector.tensor_scalar(out=soh[:], in0=so[:], scalar1=0.5,
                                        scalar2=None, op0=ALU.mult)
                nc.sync.dma_start(
                    hn_d[0::2, :].rearrange("(a b) h -> b a h", a=2), seh[:])
                nc.sync.dma_start(
                    hn_d[1::2, :].rearrange("(a b) h -> b a h", a=2), soh[:])
                nc.gpsimd.collective_compute(
                    "AllGather", ALU.bypass, replica_groups=RG,
                    ins=[hn_d[:].opt()], outs=[hnfull_d[:].opt()])

                # ---- B2: global aggregation (dst-owned, batched gather) ----
                aggn = psagg.tile([P, GW, H], F32, space="PSUM", tag="psagg")
                for k in range(NWIN_G):
                    ntk = NTG[k]
                    o0 = int(OFFG[k])
                    gath = gp.tile([P, ntk, P], BF16, tag="g_gath")
                    for j in range(ntk):
                        nc.gpsimd.indirect_dma_start(
                            out=gath[:, j, :], out_offset=None,
                            in_=hnfull_d[:],
                            in_offset=bass.IndirectOffsetOnAxis(
                                ap=gidx_sb[:, o0 + j:o0 + j + 1], axis=0))
                    gtohc = sb.tile([32, ntk * P], BF16, tag="g_toh")
                    nc.sync.dma_start(gtohc[:],
                                      gtoh[:, o0 * P:(o0 + ntk) * P])
                    gdstc = sb.tile([P, ntk * P], BF16, tag="g_dst")
                    nc.sync.dma_start(gdstc[:],
                                      gdstoh[:, o0 * P:(o0 + ntk) * P])
                    for b0 in range(0, ntk, 4):
                        bn = min(4, ntk - b0)
                        mps = ps.tile([P, GW, H], F32, space="PSUM",
                                      tag="psmsg")
                        for i in range(bn):
                            t = b0 + i
                            nc.tensor.matmul(out=mps[:, i, :],
                                             lhsT=gtohc[:, t * P:(t + 1) * P],
                                             rhs=bond_sb[:],
                                             start=True, stop=True)
                        ms = sb.tile([P, GW, H], F32, tag="g_ms")
                        nc.vector.tensor_tensor(
                            out=ms[:, :bn, :],
                            in0=gath[:, b0:b0 + bn, :],
                            in1=mps[:, :bn, :], op=ALU.add)
                        msgb = sb.tile([P, GW, H], BF16, tag="g_msg")
                        nc.scalar.activation(msgb[:, :bn, :],
                                             ms[:, :bn, :], AF.Relu)
                        for i in range(bn):
                            t = b0 + i
                            nc.tensor.matmul(out=aggn[:, k, :],
                                             lhsT=gdstc[:, t * P:(t + 1) * P],
                                             rhs=msgb[:, i, :],
                                             start=(b0 + i == 0),
                                             stop=(b0 + i == ntk - 1))

                # hlinN = (1+eps_g)*hn + aggN  (our 512 rows)
                for j in range(NODES // P):
                    hn = sb.tile([P, H], BF16, tag="b2h")
                    nc.sync.dma_start(hn[:], hn_d[j * P:(j + 1) * P, :])
                    t0 = sb.tile([P, H], F32, tag="b2t")
                    nc.vector.tensor_scalar(out=t0[:], in0=hn[:],
                                            scalar1=1.0 + eps_g[li],
                                            scalar2=None, op0=ALU.mult)
                    hl = sb.tile([P, H], BF16, tag="b2l")
                    nc.vector.tensor_tensor(out=hl[:], in0=aggn[:, j, :],
                                            in1=t0[:], op=ALU.add)
                    nc.sync.dma_start(hlinN_d[j * P:(j + 1) * P, :], hl[:])

                # global MLP on 512 rows (feature-major, one tile)
                hT = sb.tile([H, NODES], BF16, tag="n_hT")
                nc.sync.dma_start_transpose(hT[:], hlinN_d[:, :])
                wg1 = sb.tile([H, H], BF16, tag="wg1")
                nc.sync.dma_start(wg1[:], gw1[wof:wof + H, :])
                p1 = ps2.tile([H, NODES], F32, space="PSUM", tag="ps512")
                nc.tensor.matmul(out=p1[:], lhsT=wg1[:], rhs=hT[:],
                                 start=True, stop=True)
                mid = sb.tile([H, NODES], BF16, tag="n_mid")
                nc.scalar.activation(mid[:], p1[:], AF.Relu, bias=gb1c)
                wg2 = sb.tile([H, H], BF16, tag="wg2")
                nc.sync.dma_start(wg2[:], gw2[wof:wof + H, :])
                p2 = ps2.tile([H, NODES], F32, space="PSUM", tag="ps512")
                nc.tensor.matmul(out=p2[:], lhsT=wg2[:], rhs=mid[:],
                                 start=True, stop=True)
                hcr = sb.tile([H, NODES], BF16, tag="n_hcr")
                nc.scalar.activation(hcr[:], p2[:], AF.Relu, bias=gb2c)
                # BN-global stats (local slice) + allreduce
                stg = sb.tile([P, 2], F32, tag="stg")
                nc.vector.tensor_reduce(out=stg[:, 0:1], in_=hcr[:],
                                        axis=mybir.AxisListType.X, op=ALU.add)
                sqg_scr = sb.tile([H, NODES], F32, tag="n_sq")
                nc.scalar.activation(sqg_scr[:], hcr[:], AF.Square,
                                     accum_out=stg[:, 1:2])
                nc.sync.dma_start(statg_in[:], stg[:])
                nc.gpsimd.collective_compute(
                    "AllReduce", ALU.add, replica_groups=RG,
                    ins=[statg_in[:].opt()], outs=[statg_out[:].opt()])
                sgo = sb.tile([P, 2], F32, tag="sgo")
                nc.sync.dma_start(sgo[:], statg_out[:])
                mug = sb.tile([P, 1], F32, tag="mug")
                nc.vector.tensor_scalar(out=mug[:], in0=sgo[:, 0:1],
                                        scalar1=1.0 / Nn, scalar2=None,
                                        op0=ALU.mult)
                varg = sb.tile([P, 1], F32, tag="varg")
                nc.vector.tensor_tensor(out=varg[:], in0=mug[:], in1=mug[:],
                                        op=ALU.mult)
                v2g = sb.tile([P, 1], F32, tag="v2g")
                nc.vector.tensor_scalar(out=v2g[:], in0=sgo[:, 1:2],
                                        scalar1=1.0 / Nn, scalar2=None,
                                        op0=ALU.mult)
                nc.vector.tensor_tensor(out=varg[:], in0=v2g[:], in1=varg[:],
                                        op=ALU.subtract)
                nc.vector.tensor_scalar(out=varg[:], in0=varg[:], scalar1=1e-5,
                                        scalar2=None, op0=ALU.add)
                sdg = sb.tile([P, 1], F32, tag="sdg")
                nc.scalar.activation(sdg[:], varg[:], AF.Sqrt)
                rsg = sb.tile([P, 1], F32, tag="rsg")
                nc.vector.reciprocal(rsg[:], sdg[:])
                ag_bn = sb.tile([P, 1], F32, tag="ag_bn")
                nc.vector.tensor_tensor(out=ag_bn[:], in0=gbngc, in1=rsg[:],
                                        op=ALU.mult)
                nbg = sb.tile([P, 1], F32, tag="nbg")
                nc.vector.tensor_tensor(out=nbg[:], in0=mug[:], in1=ag_bn[:],
                                        op=ALU.mult)
                bg_bn = sb.tile([P, 1], F32, tag="bg_bn")
                nc.vector.tensor_tensor(out=bg_bn[:], in0=gbnbc, in1=nbg[:],
                                        op=ALU.subtract)
                # h_node_new^T = hn^T + BN(hcr)
                hnT = sb.tile([H, NODES], BF16, tag="n_hnT")
                nc.sync.dma_start_transpose(hnT[:], hn_d[:, :])
                hcb = sb.tile([H, NODES], F32, tag="n_hcb")
                nc.vector.tensor_scalar(out=hcb[:], in0=hcr[:],
                                        scalar1=ag_bn[:], scalar2=bg_bn[:],
                                        op0=ALU.mult, op1=ALU.add)
                hnn = sb.tile([H, NODES], BF16, tag="n_hnn")
                nc.vector.tensor_tensor(out=hnn[:], in0=hcb[:], in1=hnT[:],
                                        op=ALU.add)
                # hb^T = bcast_w^T @ hnn^T
                wbc = sb.tile([H, H], BF16, tag="wbc")
                nc.sync.dma_start(wbc[:], bcw[wof:wof + H, :])
                p3 = ps2.tile([H, NODES], F32, space="PSUM", tag="ps512")
                nc.tensor.matmul(out=p3[:], lhsT=wbc[:], rhs=hnn[:],
                                 start=True, stop=True)
                hbT = sbw.tile([H, NODES], BF16, tag="hbT")
                nc.vector.tensor_copy(out=hbT[:], in_=p3[:])

                # ---- B3/B4: cat MLP + LN + residual ----
                wc1t = sbw.tile([H, H], BF16, tag="wc1t")
                nc.sync.dma_start(wc1t[:], cw1t[wof:wof + H, :])
                wc1b = sbw.tile([H, H], BF16, tag="wc1b")
                nc.sync.dma_start(wc1b[:], cw1b[wof:wof + H, :])
                wc2 = sbw.tile([H, H], BF16, tag="wc2")
                nc.sync.dma_start(wc2[:], cw2[wof:wof + H, :])
                cb2 = sbw.tile([P, H], F32, tag="cb2")
                nc.sync.dma_start(cb2[:], cb2rep[li * P:(li + 1) * P, :])
                lng = sbw.tile([P, H], F32, tag="lng")
                nc.sync.dma_start(lng[:], lngrep[li * P:(li + 1) * P, :])
                lnb = sbw.tile([P, H], F32, tag="lnb")
                nc.sync.dma_start(lnb[:], lnbrep[li * P:(li + 1) * P, :])
                for rt in range(ROWS // 512):
                    hrt = sb.tile([H, 512], BF16, tag="c_hrt")
                    nc.sync.dma_start(hrt[:],
                                      hrT_d[:, rt * 512:(rt + 1) * 512])
                    hbn = sb.tile([H, 512], BF16, tag="c_hbn")
                    nc.vector.tensor_scalar(out=hbn[:], in0=hrt[:],
                                            scalar1=a_bn[:], scalar2=b_bn[:],
                                            op0=ALU.mult, op1=ALU.add)
                    pc = ps2.tile([H, 512], F32, space="PSUM", tag="ps512")
                    nc.tensor.matmul(out=pc[:], lhsT=wc1t[:], rhs=hbn[:],
                                     start=True, stop=False)
                    hbe = hbT[:, rt * 8:(rt + 1) * 8, None]
                    nc.tensor.matmul(out=pc[:], lhsT=wc1b[:],
                                     rhs=hbe.to_broadcast([H, 8, 64]),
                                     start=False, stop=True)
                    mid2 = sb.tile([H, 512], BF16, tag="c_mid2")
                    nc.scalar.activation(mid2[:], pc[:], AF.Gelu, bias=catb1c)
                    pn = ps2.tile([P, 4, H], F32, space="PSUM", tag="ps512")
                    for j in range(4):
                        nc.tensor.matmul(out=pn[:, j, :],
                                         lhsT=mid2[:, j * P:(j + 1) * P],
                                         rhs=wc2[:], start=True, stop=True)
                    xn = sb.tile([P, 4, H], F32, tag="c_xn")
                    nc.vector.tensor_tensor(
                        out=xn[:], in0=pn[:],
                        in1=cb2[:, None, :].to_broadcast([P, 4, H]),
                        op=ALU.add)
                    mu4 = sb.tile([P, 4], F32, tag="c_mu4")
                    nc.vector.tensor_reduce(out=mu4[:], in_=xn[:],
                                            axis=mybir.AxisListType.X,
                                            op=ALU.add)
                    nc.vector.tensor_scalar(out=mu4[:], in0=mu4[:],
                                            scalar1=1.0 / H, scalar2=None,
                                            op0=ALU.mult)
                    sq4 = sb.tile([P, 4, H], F32, tag="c_sq4")
                    nc.vector.tensor_tensor(out=sq4[:], in0=xn[:], in1=xn[:],
                                            op=ALU.mult)
                    s24 = sb.tile([P, 4], F32, tag="c_s24")
                    nc.vector.tensor_reduce(out=s24[:], in_=sq4[:],
                                            axis=mybir.AxisListType.X,
                                            op=ALU.add)
                    nc.vector.tensor_scalar(out=s24[:], in0=s24[:],
                                            scalar1=1.0 / H, scalar2=None,
                                            op0=ALU.mult)
                    m2 = sb.tile([P, 4], F32, tag="c_m2")
                    nc.vector.tensor_tensor(out=m2[:], in0=mu4[:], in1=mu4[:],
                                            op=ALU.mult)
                    nc.vector.tensor_tensor(out=s24[:], in0=s24[:], in1=m2[:],
                                            op=ALU.subtract)
                    nc.vector.tensor_scalar(out=s24[:], in0=s24[:],
                                            scalar1=1e-5, scalar2=None,
                                            op0=ALU.add)
                    sd4 = sb.tile([P, 4], F32, tag="c_sd4")
                    nc.scalar.activation(sd4[:], s24[:], AF.Sqrt)
                    rs4 = sb.tile([P, 4], F32, tag="c_rs4")
                    nc.vector.reciprocal(rs4[:], sd4[:])
                    nc.vector.tensor_tensor(
                        out=xn[:], in0=xn[:],
                        in1=mu4[:, :, None].to_broadcast([P, 4, H]),
                        op=ALU.subtract)
                    nc.vector.tensor_tensor(
                        out=xn[:], in0=xn[:],
                        in1=rs4[:, :, None].to_broadcast([P, 4, H]),
                        op=ALU.mult)
                    nc.vector.tensor_tensor(
                        out=xn[:], in0=xn[:],
                        in1=lng[:, None, :].to_broadcast([P, 4, H]),
                        op=ALU.mult)
                    nc.vector.tensor_tensor(
                        out=xn[:], in0=xn[:],
                        in1=lnb[:, None, :].to_broadcast([P, 4, H]),
                        op=ALU.add)
                    hin = sb.tile([P, 4, H], BF16, tag="c_hin")
                    nc.sync.dma_start(hin[:],
                                      h_pm[:, rt * 4:(rt + 1) * 4, :])
                    nc.vector.tensor_tensor(out=xn[:], in0=xn[:], in1=hin[:],
                                            op=ALU.add)
                    hout = sb.tile([P, 4, H], BF16, tag="c_hout")
                    vsl = validf_sb[:, rt * 4:(rt + 1) * 4, None]
                    nc.vector.tensor_tensor(
                        out=hout[:], in0=xn[:],
                        in1=vsl.to_broadcast([P, 4, H]), op=ALU.mult)
                    nc.sync.dma_start(h_pm[:, rt * 4:(rt + 1) * 4, :],
                                      hout[:])

            # ================= pooling =================
            pp = pspool.tile([NG, H], F32, space="PSUM", tag="poolps")
            for g in range(NWIN // GW):
                htile = sb.tile([P, GW, H], BF16, tag="p_h")
                nc.sync.dma_start(htile[:], h_pm[:, g * GW:(g + 1) * GW, :])
                wp = sb.tile([P, GW * NG], BF16, tag="p_w")
                nc.sync.dma_start(wp[:],
                                  wpool[:, g * GW * NG:(g + 1) * GW * NG])
                for k in range(GW):
                    rt = g * GW + k
                    nc.tensor.matmul(out=pp[:],
                                     lhsT=wp[:, k * NG:(k + 1) * NG],
                                     rhs=htile[:, k, :],
                                     start=(rt == 0), stop=(rt == NWIN - 1))
            po = sb.tile([NG, H], F32, tag="p_o")
            nc.vector.tensor_copy(out=po[:], in_=pp[:])
            nc.sync.dma_start(pool_in[:], po[:])
            nc.gpsimd.collective_compute(
                "AllReduce", ALU.add, replica_groups=RG,
                ins=[pool_in[:].opt()], outs=[pool_out[:].opt()])
            fo = sb.tile([NG, H], F32, tag="p_f")
            nc.sync.dma_start(fo[:], pool_out[:])
            nc.sync.dma_start(out_ext[:], fo[:])

    nc.finalize()
    return nc


# ----------------------------------------------------------------------------
# kernel entry
# ----------------------------------------------------------------------------

def kernel(**inputs):
    atom_emb = np.asarray(inputs["atom_emb"], np.float32)
    bond_emb = np.asarray(inputs["bond_emb"], np.float32)
    rwse_w = np.asarray(inputs["rwse_w"], np.float32)
    rwse_b = np.asarray(inputs["rwse_b"], np.float32)
    rwse = np.asarray(inputs["rwse"], np.float32)
    l_eps = np.asarray(inputs["l_eps"], np.float32)
    l_w1 = np.asarray(inputs["l_w1"], np.float32)
    l_b1 = np.asarray(inputs["l_b1"], np.float32)
    l_w2 = np.asarray(inputs["l_w2"], np.float32)
    l_b2 = np.asarray(inputs["l_b2"], np.float32)
    l_bng = np.asarray(inputs["l_bng"], np.float32)
    l_bnb = np.asarray(inputs["l_bnb"], np.float32)
    g_eps = np.asarray(inputs["g_eps"], np.float32)
    g_w1 = np.asarray(inputs["g_w1"], np.float32)
    g_b1 = np.asarray(inputs["g_b1"], np.float32)
    g_w2 = np.asarray(inputs["g_w2"], np.float32)
    g_b2 = np.asarray(inputs["g_b2"], np.float32)
    g_bng = np.asarray(inputs["g_bng"], np.float32)
    g_bnb = np.asarray(inputs["g_bnb"], np.float32)
    bcast_w = np.asarray(inputs["bcast_w"], np.float32)
    cat_w1 = np.asarray(inputs["cat_w1"], np.float32)
    cat_b1 = np.asarray(inputs["cat_b1"], np.float32)
    cat_w2 = np.asarray(inputs["cat_w2"], np.float32)
    cat_b2 = np.asarray(inputs["cat_b2"], np.float32)
    ln_g = np.asarray(inputs["ln_g"], np.float32)
    ln_b = np.asarray(inputs["ln_b"], np.float32)
    x_ids = np.asarray(inputs["x_ids"], np.int64)
    intra_ei = np.asarray(inputs["intra_ei"], np.int64)
    intra_ea_ids = np.asarray(inputs["intra_ea_ids"], np.int64)
    global_ei = np.asarray(inputs["global_ei"], np.int64)
    global_ea_ids = np.asarray(inputs["global_ea_ids"], np.int64)
    node_ids = np.asarray(inputs["node_ids"], np.int64)
    valid = np.asarray(inputs["valid"], np.int64)
    batch = np.asarray(inputs["batch"], np.int64)

    bond_ext = np.zeros((32, H), np.float32)
    bond_ext[:16] = bond_emb
    rww = np.zeros((32, H), np.float32)
    rww[:16] = rwse_w
    rww[16] = rwse_b

    # ---- per-core intra edge split + tile counts ----
    esrc, edst = intra_ei[0], intra_ei[1]
    ecore = (edst // K_SUB) // SUBS
    per_core = []
    win_counts = []
    for c in range(NCORES):
        m = ecore == c
        s = esrc[m] - c * ROWS
        d = edst[m] - c * ROWS
        t = intra_ea_ids[m]
        per_core.append((s, d, t))
        win_counts.append(np.bincount(d // P, minlength=NWIN))
    NT = np.ones(NWIN, np.int64)
    for cnt in win_counts:
        NT = np.maximum(NT, (cnt + P - 1) // P)
    OFF = np.concatenate([[0], np.cumsum(NT)]).astype(np.int64)

    # ---- per-core global edge split (dst-window-owned) ----
    gsrc_, gdst_ = global_ei[0], global_ei[1]
    gcore = gdst_ // NODES
    gper_core = []
    gwin_counts = []
    for c in range(NCORES):
        m = gcore == c
        gper_core.append((gsrc_[m], gdst_[m], global_ea_ids[m]))
        gwin_counts.append(
            np.bincount((gdst_[m] - c * NODES) // P, minlength=NWIN_G))
    NTG = np.ones(NWIN_G, np.int64)
    for cnt in gwin_counts:
        NTG = np.maximum(NTG, (cnt + P - 1) // P)
    OFFG = np.concatenate([[0], np.cumsum(NTG)]).astype(np.int64)

    # ---- pooling weights per core ----
    valid_f = valid.astype(np.float32)
    cnt_s = valid_f.reshape(S_TOT, K_SUB).sum(1)
    wrow = 1.0 / (2.0 * np.maximum(cnt_s, 1.0))       # per subgraph
    node_of_sub = np.arange(S_TOT) // M_SUB
    graph_of_sub = batch[node_of_sub]                  # [S_TOT]

    nc = build_program(list(NT), list(NTG), [float(x) for x in l_eps],
                       [float(x) for x in g_eps])

    bias_cols = np.zeros((P, 8 * L), np.float32)
    bias2_cols = np.zeros((P, 4 * L), np.float32)
    for li in range(L):
        bias_cols[:, 8 * li + 0] = l_b1[li]
        bias_cols[:, 8 * li + 1] = l_b2[li]
        bias_cols[:, 8 * li + 2] = g_b1[li]
        bias_cols[:, 8 * li + 3] = g_b2[li]
        bias_cols[:, 8 * li + 4] = cat_b1[li]
        bias_cols[:, 8 * li + 5] = l_bng[li]
        bias_cols[:, 8 * li + 6] = l_bnb[li]
        bias_cols[:, 8 * li + 7] = g_bng[li]
        bias2_cols[:, 4 * li] = g_bnb[li]

    in_maps = []
    for c in range(NCORES):
        r0 = c * ROWS
        d = {}
        d["atom"] = atom_emb.astype(BF)
        d["bond"] = bond_ext.astype(BF)
        d["rww"] = rww.astype(BF)
        # h0 tables (valid-masked)
        vloc = valid_f[r0:r0 + ROWS]
        xids = x_ids[r0:r0 + ROWS]
        nids = node_ids[r0:r0 + ROWS]
        xoh = np.zeros((P, ROWS), BF)
        vmask = vloc > 0
        cols = np.arange(ROWS)
        xoh[xids[vmask], cols[vmask]] = 1.0
        rw17 = np.zeros((32, ROWS), np.float32)
        rw17[:16, vmask] = rwse[nids[vmask]].T
        rw17[16, vmask] = 1.0
        d["xoh"] = xoh
        d["rw17"] = rw17.astype(BF)
        # intra tables ([selT | dstoh] interleaved per tile)
        s_, d_, t_ = per_core[c]
        selT, toh_t, dstoh_t = build_intra_tables(s_, d_, t_, NT, OFF)
        TTc = selT.shape[1] // P
        tabs = np.empty((P, TTc, 2, P), BF)
        tabs[:, :, 0, :] = selT.reshape(P, TTc, P)
        tabs[:, :, 1, :] = dstoh_t.reshape(P, TTc, P)
        d["tabs"] = np.ascontiguousarray(tabs.reshape(P, TTc * 2 * P))
        d["tohi"] = toh_t
        # global tables
        gs_, gd_, gt_ = gper_core[c]
        gsrc_flat, gtoh_t, gdstoh_t = build_global_tables(
            gs_, gd_, gt_, NTG, OFFG, c * NODES)
        d["gtoh"] = gtoh_t
        d["gdstoh"] = gdstoh_t
        d["gidx"] = np.ascontiguousarray(
            gsrc_flat.reshape(-1, P).T).astype(np.int32)
        d["wl1"] = l_w1.reshape(L * H, H).astype(BF)
        d["wl2"] = l_w2.reshape(L * H, H).astype(BF)
        d["gw1"] = g_w1.reshape(L * H, H).astype(BF)
        d["gw2"] = g_w2.reshape(L * H, H).astype(BF)
        d["bcw"] = bcast_w.reshape(L * H, H).astype(BF)
        d["cw1t"] = cat_w1[:, :H, :].reshape(L * H, H).astype(BF)
        d["cw1b"] = cat_w1[:, H:, :].reshape(L * H, H).astype(BF)
        d["cw2"] = cat_w2.reshape(L * H, H).astype(BF)
        d["bias_cols"] = bias_cols
        d["bias2_cols"] = bias2_cols
        d["cb2rep"] = np.repeat(cat_b2[:, None, :], P, 1).reshape(L * P, H).astype(np.float32)
        d["lngrep"] = np.repeat(ln_g[:, None, :], P, 1).reshape(L * P, H).astype(np.float32)
        d["lnbrep"] = np.repeat(ln_b[:, None, :], P, 1).reshape(L * P, H).astype(np.float32)
        d["validf"] = np.ascontiguousarray(vloc.reshape(NWIN, P).T)
        wp = np.zeros((ROWS, NG), np.float32)
        for s in range(SUBS):
            gs = c * SUBS + s
            wp[s * K_SUB:(s + 1) * K_SUB, graph_of_sub[gs]] = wrow[gs]
        d["wpool"] = np.ascontiguousarray(
            wp.reshape(NWIN, P, NG).transpose(1, 0, 2).reshape(P, NWIN * NG)).astype(BF)
        in_maps.append(d)

    if os.environ.get("BASS_KERNEL_SIM"):
        from concourse.bass_interp import MultiCoreSim
        sim = MultiCoreSim(nc, num_cores=NCORES,
                           num_workers=int(os.environ.get("BASS_SIM_WORKERS", "8")))
        for c in range(NCORES):
            cs = sim.cores.get(c)
            if cs is None:
                continue
            for name, val in in_maps[c].items():
                view = cs.tensor(name)
                view[:] = val
        sim.simulate()
        out = sim.cores[0].tensor("out") if 0 in sim.cores else sim.outs[0]["out"]
        kernel.last_exec_ns = None
        return np.asarray(out, np.float32)

    res = run_bass_kernel_spmd(nc, in_maps, list(range(NCORES)),
                               **_extra_run_kwargs())
    out = res.results[0]["out"]
    kernel.last_exec_ns = res.exec_time_ns
    return np.asarray(out, np.float32)


def _extra_run_kwargs():
    kw = {}
    if os.environ.get("BASS_KERNEL_TRACE"):
        kw["trace"] = True
    return kw


kernel.last_exec_ns = None


# revision 21
# speedup vs baseline: 2.0726x; 1.1505x over previous
"""Trainium2 distributed Bass kernel for the hierarchical GNN encoder.

Strategy (8 NeuronCores, SPMD):
  - Shard the S=8192 subgraphs contiguously: 1024 subgraphs (=32768 flat rows,
    512 original nodes) per core.  Intra edges are subgraph-local so each
    core's intra edges are fully local, and src/dst of every edge live in the
    same 128-row window (4 subgraphs).
  - NO indirect row gathers for the intra graph: messages are computed as
    PE matmuls against host-precomputed one-hot selection tables:
      msg  = selT^T @ h_window + toh^T @ bond      (relu on Scalar engine)
      agg += dstoh^T @ msg                          (PSUM accumulation)
    Edges are dst-sorted per window and cut into 128-edge tiles with no
    run/packing constraints (duplicate dst rows accumulate in PSUM).
  - h0 init is also matmul-only: atom one-hot table and a host-gathered
    rwse[node_ids] table (with a ones row folding in the bias).
  - Global graph: edges are owned by the dst-window owner (no ReduceScatter);
    h_node[src] rows come from per-tile indirect gathers out of the
    all-gathered h_node table (68 gathers/layer vs 6000+ in the old design).
  - MLPs run on PE with weights stationary; BatchNorm batch stats are
    all-reduced ([H,2] per norm); the final pooled [64,H] output is
    all-reduced.
"""

import math
import os
import sys

sys.path.insert(0, "/opt/trn_rl_repo")

import numpy as np
import ml_dtypes

from concourse import bacc, bass, mybir, tile
from concourse.bass_utils import run_bass_kernel_spmd

P = 128
H = 128
L = 4
NCORES = 8
NG = 64
Nn = 4096
M_SUB = 2          # subgraphs per node
K_SUB = 32         # nodes per subgraph
S_TOT = Nn * M_SUB
SK = S_TOT * K_SUB
ROWS = SK // NCORES          # 32768 flat rows per core
SUBS = S_TOT // NCORES       # 1024 subgraphs per core
NODES = Nn // NCORES         # 512 nodes per core
NWIN = ROWS // P             # 256 agg windows per core
NWIN_G = NODES // P          # 4 global dst windows owned per core
F32 = mybir.dt.float32
BF16 = mybir.dt.bfloat16
I16 = mybir.dt.int16
I32 = mybir.dt.int32
AF = mybir.ActivationFunctionType
ALU = mybir.AluOpType
BF = ml_dtypes.bfloat16
GW = 4             # windows per intra group


# ----------------------------------------------------------------------------
# Host-side edge table construction
# ----------------------------------------------------------------------------

def tile_counts(dst_sorted_win, n_win, n_cores_lists):
    """Per-window tile counts (max across cores)."""
    nt = np.ones(n_win, np.int64)
    for counts in n_cores_lists:
        nt = np.maximum(nt, (counts + P - 1) // P)
    return nt


def build_intra_tables(s, d, t, nt, off):
    """One-hot tables for a core's intra edges (dst-sorted within window).

    s, d: row ids relative to core (0..ROWS); t: bond type.
    nt: per-window tile count (shared across cores); off: prefix offsets.
    Returns selT [128, TT*128], toh [32, TT*128], dstoh [128, TT*128] (bf16).
    """
    TT = int(off[-1])
    w = d // P
    order = np.argsort(w, kind="stable")
    s, d, t, w = s[order], d[order], t[order], w[order]
    # position within window
    cnt = np.bincount(w, minlength=len(nt))
    starts = np.concatenate([[0], np.cumsum(cnt)[:-1]])
    pos = np.arange(len(d)) - starts[w]
    col = (off[w] + pos // P) * P + pos % P
    selT = np.zeros((P, TT * P), BF)
    toh = np.zeros((32, TT * P), BF)
    dstoh = np.zeros((P, TT * P), BF)
    selT[s - w * P, col] = 1.0
    toh[t, col] = 1.0
    dstoh[pos % P, (off[w] + pos // P) * P + (d - w * P)] = 1.0
    return selT, toh, dstoh


def build_global_tables(s, d, t, ntg, offg, win_base):
    """Tables for a core's owned global edges (dst in core's 4 windows).

    Returns gsrc_flat [TTG*128] int64 (pad 0), gtoh [32, TTG*128],
    gdstoh [128, TTG*128].
    """
    TTG = int(offg[-1])
    w = (d - win_base) // P
    order = np.argsort(w, kind="stable")
    s, d, t, w = s[order], d[order], t[order], w[order]
    cnt = np.bincount(w, minlength=len(ntg))
    starts = np.concatenate([[0], np.cumsum(cnt)[:-1]])
    pos = np.arange(len(d)) - starts[w]
    col = (offg[w] + pos // P) * P + pos % P
    gsrc = np.zeros(TTG * P, np.int64)
    gtoh = np.zeros((32, TTG * P), BF)
    gdstoh = np.zeros((P, TTG * P), BF)
    gsrc[col] = s
    gtoh[t, col] = 1.0
    gdstoh[pos % P, (offg[w] + pos // P) * P + ((d - win_base) - w * P)] = 1.0
    return gsrc, gtoh, gdstoh


def wrap16_rep(idx):
    """[n] int -> [128, n//16] int16: wrapped 16-partition layout replicated
    8x across the 128 partitions (dma_gather index format)."""
    n = len(idx)
    assert n % 16 == 0
    w = np.ascontiguousarray(idx.reshape(n // 16, 16).T.astype(np.int16))
    return np.ascontiguousarray(np.tile(w, (8, 1)))


# ----------------------------------------------------------------------------
# Device program
# ----------------------------------------------------------------------------

def build_program(NT, NTG, eps_l, eps_g):
    NT = [int(x) for x in NT]
    NTG = [int(x) for x in NTG]
    OFF = np.concatenate([[0], np.cumsum(NT)]).astype(np.int64)
    OFFG = np.concatenate([[0], np.cumsum(NTG)]).astype(np.int64)
    TT = int(OFF[-1])
    TTG = int(OFFG[-1])

    nc = bacc.Bacc(None, target_bir_lowering=False, debug=True)

    def inp(name, shape, dtype):
        return nc.declare_dram_parameter(name, list(shape), dtype, isOutput=False)

    # weights / tables
    atom = inp("atom", [P, H], BF16)
    bond = inp("bond", [32, H], BF16)           # 16 real rows, rest zero
    rww = inp("rww", [32, H], BF16)             # rwse_w rows 0-15, b at row 16
    xoh = inp("xoh", [P, ROWS], BF16)           # atom one-hot (valid-masked)
    rw17 = inp("rw17", [32, ROWS], BF16)        # rwse[node_ids]^T + ones row 16
    tabs = inp("tabs", [P, TT * 2 * P], BF16)   # [selT | dstoh] per tile
    tohi = inp("tohi", [32, TT * P], BF16)
    gtoh = inp("gtoh", [32, TTG * P], BF16)
    gdstoh = inp("gdstoh", [P, TTG * P], BF16)
    gidx = inp("gidx", [P, TTG], I32)
    wl1 = inp("wl1", [L * H, H], BF16)
    wl2 = inp("wl2", [L * H, H], BF16)
    gw1 = inp("gw1", [L * H, H], BF16)
    gw2 = inp("gw2", [L * H, H], BF16)
    bcw = inp("bcw", [L * H, H], BF16)
    cw1t = inp("cw1t", [L * H, H], BF16)        # cat_w1 top half
    cw1b = inp("cw1b", [L * H, H], BF16)        # cat_w1 bottom half
    cw2 = inp("cw2", [L * H, H], BF16)
    bias_cols = inp("bias_cols", [P, 8 * L], F32)
    bias2_cols = inp("bias2_cols", [P, 4 * L], F32)
    cb2rep = inp("cb2rep", [L * P, H], F32)     # cat_b2 replicated per layer
    lngrep = inp("lngrep", [L * P, H], F32)
    lnbrep = inp("lnbrep", [L * P, H], F32)
    validf = inp("validf", [P, NWIN], F32)
    wpool = inp("wpool", [P, NWIN * NG], BF16)

    out_ext = nc.declare_dram_parameter("out", [NG, H], F32, isOutput=True)

    # internal DRAM
    h_pm = nc.dram_tensor("h_pm", [P, NWIN, H], BF16)
    hlin_d = nc.dram_tensor("hlin_d", [ROWS, H], BF16)
    hrT_d = nc.dram_tensor("hrT_d", [H, ROWS], BF16)
    hn_d = nc.dram_tensor("hn_d", [NODES, H], BF16)
    hnfull_d = nc.dram_tensor("hnfull_d", [Nn, H], BF16)
    hlinN_d = nc.dram_tensor("hlinN_d", [NODES, H], BF16)
    stat_in = nc.dram_tensor("stat_in", [P, 2], F32)
    stat_out = nc.dram_tensor("stat_out", [P, 2], F32)
    statg_in = nc.dram_tensor("statg_in", [P, 2], F32)
    statg_out = nc.dram_tensor("statg_out", [P, 2], F32)
    pool_in = nc.dram_tensor("pool_in", [NG, H], F32)
    pool_out = nc.dram_tensor("pool_out", [NG, H], F32)

    RG = [list(range(NCORES))]

    with tile.TileContext(nc) as tc:
        with (
            tc.tile_pool(name="const", bufs=1) as cpool,
            tc.tile_pool(name="sb", bufs=2) as sb,
            tc.tile_pool(name="sbw", bufs=2) as sbw,
            tc.tile_pool(name="gp", bufs=2) as gp,
            tc.tile_pool(name="ps", bufs=2, space="PSUM") as ps,
            tc.tile_pool(name="psagg", bufs=2, space="PSUM") as psagg,
            tc.tile_pool(name="ps2", bufs=2, space="PSUM") as ps2,
            tc.tile_pool(name="pspool", bufs=1, space="PSUM") as pspool,
        ):
            # ---- constants resident in SBUF ----
            bias_sb = cpool.tile([P, 8 * L], F32)
            nc.sync.dma_start(bias_sb[:], bias_cols[:])
            bias2_sb = cpool.tile([P, 4 * L], F32)
            nc.sync.dma_start(bias2_sb[:], bias2_cols[:])
            validf_sb = cpool.tile([P, NWIN], F32)
            nc.sync.dma_start(validf_sb[:], validf[:])
            bond_sb = cpool.tile([32, H], BF16)
            nc.sync.dma_start(bond_sb[:], bond[:])
            atom_sb = cpool.tile([P, H], BF16)
            nc.sync.dma_start(atom_sb[:], atom[:])
            rww_sb = cpool.tile([32, H], BF16)
            nc.sync.dma_start(rww_sb[:], rww[:])
            gidx_sb = cpool.tile([P, TTG], I32)
            nc.sync.dma_start(gidx_sb[:], gidx[:])

            # ================= init: h0 = atomsel + relu(rwsesel@W + b) =====
            for g in range(NWIN // GW):
                c0, c1 = g * GW * P, (g + 1) * GW * P
                xo = sb.tile([P, GW, P], BF16, tag="h0xo")
                nc.sync.dma_start(xo[:], xoh[:, c0:c1])
                rw = sb.tile([32, GW, P], BF16, tag="h0rw")
                nc.sync.dma_start(rw[:], rw17[:, c0:c1])
                psa = ps.tile([P, GW, H], F32, space="PSUM", tag="psmsg")
                psr = psagg.tile([P, GW, H], F32, space="PSUM", tag="psagg")
                for j in range(GW):
                    nc.tensor.matmul(out=psa[:, j, :], lhsT=xo[:, j, :],
                                     rhs=atom_sb[:], start=True, stop=True)
                    nc.tensor.matmul(out=psr[:, j, :], lhsT=rw[:, j, :],
                                     rhs=rww_sb[:], start=True, stop=True)
                rr = sb.tile([P, GW, H], BF16, tag="h0rr")
                nc.scalar.activation(rr[:], psr[:], AF.Relu)
                h0 = sb.tile([P, GW, H], BF16, tag="h0h")
                nc.vector.tensor_tensor(out=h0[:], in0=psa[:], in1=rr[:],
                                        op=ALU.add)
                nc.sync.dma_start(h_pm[:, g * GW:(g + 1) * GW, :], h0[:])

            # ================= layers =================
            for li in range(L):
                wof = li * H
                b1c = bias_sb[:, 8 * li + 0:8 * li + 1]
                b2c = bias_sb[:, 8 * li + 1:8 * li + 2]
                gb1c = bias_sb[:, 8 * li + 2:8 * li + 3]
                gb2c = bias_sb[:, 8 * li + 3:8 * li + 4]
                catb1c = bias_sb[:, 8 * li + 4:8 * li + 5]
                bngc = bias_sb[:, 8 * li + 5:8 * li + 6]
                bnbc = bias_sb[:, 8 * li + 6:8 * li + 7]
                gbngc = bias_sb[:, 8 * li + 7:8 * li + 8]
                gbnbc = bias2_sb[:, 4 * li:4 * li + 1]

                # ---- A: intra aggregation + hlin ----
                for g in range(NWIN // GW):
                    nt_k = NT[g * GW:(g + 1) * GW]
                    ntg = sum(nt_k)
                    o0 = int(OFF[g * GW])
                    tabc = sb.tile([P, ntg, 2, P], BF16, tag="a_tab", bufs=3)
                    nc.sync.dma_start(tabc[:],
                                      tabs[:, o0 * 2 * P:(o0 + ntg) * 2 * P])
                    tohc = sb.tile([32, ntg * P], BF16, tag="a_toh", bufs=3)
                    nc.sync.dma_start(tohc[:],
                                      tohi[:, o0 * P:(o0 + ntg) * P])
                    hw = sb.tile([P, GW, H], BF16, tag="a_hw", bufs=3)
                    nc.sync.dma_start(hw[:], h_pm[:, g * GW:(g + 1) * GW, :])
                    agg = psagg.tile([P, GW, H], F32, space="PSUM", tag="psagg")
                    tlist = [(k, j) for k in range(GW) for j in range(nt_k[k])]
                    for b0 in range(0, ntg, 4):
                        bl = tlist[b0:b0 + 4]
                        mps = ps.tile([P, GW, H], F32, space="PSUM", tag="psmsg")
                        for i, (k, j) in enumerate(bl):
                            t = b0 + i
                            nc.tensor.matmul(out=mps[:, i, :],
                                             lhsT=tabc[:, t, 0, :],
                                             rhs=hw[:, k, :],
                                             start=True, stop=False)
                            nc.tensor.matmul(out=mps[:, i, :],
                                             lhsT=tohc[:, t * P:(t + 1) * P],
                                             rhs=bond_sb[:],
                                             start=False, stop=True)
                        msg = sb.tile([P, GW, H], BF16, tag="a_msg")
                        nc.scalar.activation(msg[:, :len(bl), :],
                                             mps[:, :len(bl), :], AF.Relu)
                        for i, (k, j) in enumerate(bl):
                            t = b0 + i
                            nc.tensor.matmul(out=agg[:, k, :],
                                             lhsT=tabc[:, t, 1, :],
                                             rhs=msg[:, i, :],
                                             start=(j == 0),
                                             stop=(j == nt_k[k] - 1))
                    htf = sb.tile([P, GW, H], F32, tag="a_htf")
                    nc.vector.tensor_scalar(out=htf[:], in0=hw[:],
                                            scalar1=1.0 + eps_l[li],
                                            scalar2=None, op0=ALU.mult)
                    hl = sb.tile([P, GW, H], BF16, tag="a_hl")
                    nc.vector.tensor_tensor(out=hl[:], in0=agg[:], in1=htf[:],
                                            op=ALU.add)
                    nc.sync.dma_start(
                        hlin_d[g * GW * P:(g + 1) * GW * P, :].rearrange(
                            "(a b) h -> b a h", a=GW), hl[:])

                # ---- A2: local MLP (feature-major) + BN stats ----
                sxc = sbw.tile([P, ROWS // 512], F32, tag="sxc")
                sqc = sbw.tile([P, ROWS // 512], F32, tag="sqc")
                w1 = sbw.tile([H, H], BF16, tag="w1")
                nc.sync.dma_start(w1[:], wl1[wof:wof + H, :])
                w2 = sbw.tile([H, H], BF16, tag="w2")
                nc.sync.dma_start(w2[:], wl2[wof:wof + H, :])
                for rt in range(ROWS // 512):
                    hT = sb.tile([H, 512], BF16, tag="m_hT")
                    nc.sync.dma_start_transpose(
                        hT[:], hlin_d[rt * 512:(rt + 1) * 512, :])
                    p1 = ps2.tile([H, 512], F32, space="PSUM", tag="ps512")
                    nc.tensor.matmul(out=p1[:], lhsT=w1[:], rhs=hT[:],
                                     start=True, stop=True)
                    mid = sb.tile([H, 512], BF16, tag="m_mid")
                    nc.scalar.activation(mid[:], p1[:], AF.Relu, bias=b1c)
                    p2 = ps2.tile([H, 512], F32, space="PSUM", tag="ps512")
                    nc.tensor.matmul(out=p2[:], lhsT=w2[:], rhs=mid[:],
                                     start=True, stop=True)
                    hr = sb.tile([H, 512], BF16, tag="m_hr")
                    nc.scalar.activation(hr[:], p2[:], AF.Relu, bias=b2c)
                    nc.vector.tensor_reduce(
                        out=sxc[:, rt:rt + 1], in_=hr[:],
                        axis=mybir.AxisListType.X, op=ALU.add)
                    sq_scr = sb.tile([H, 512], F32, tag="m_sq")
                    nc.scalar.activation(sq_scr[:], hr[:], AF.Square,
                                         accum_out=sqc[:, rt:rt + 1])
                    nc.sync.dma_start(hrT_d[:, rt * 512:(rt + 1) * 512], hr[:])

                # ---- BN local stats allreduce ----
                st = sb.tile([P, 2], F32, tag="st")
                nc.vector.tensor_reduce(out=st[:, 0:1], in_=sxc[:],
                                        axis=mybir.AxisListType.X, op=ALU.add)
                nc.vector.tensor_reduce(out=st[:, 1:2], in_=sqc[:],
                                        axis=mybir.AxisListType.X, op=ALU.add)
                nc.sync.dma_start(stat_in[:], st[:])
                nc.gpsimd.collective_compute(
                    "AllReduce", ALU.add, replica_groups=RG,
                    ins=[stat_in[:].opt()], outs=[stat_out[:].opt()])
                sg = sb.tile([P, 2], F32, tag="sg")
                nc.sync.dma_start(sg[:], stat_out[:])
                mu = sb.tile([P, 1], F32, tag="mu")
                nc.vector.tensor_scalar(out=mu[:], in0=sg[:, 0:1],
                                        scalar1=1.0 / SK, scalar2=None,
                                        op0=ALU.mult)
                var = sb.tile([P, 1], F32, tag="var")
                nc.vector.tensor_tensor(out=var[:], in0=mu[:], in1=mu[:],
                                        op=ALU.mult)
                v2 = sb.tile([P, 1], F32, tag="v2")
                nc.vector.tensor_scalar(out=v2[:], in0=sg[:, 1:2],
                                        scalar1=1.0 / SK, scalar2=None,
                                        op0=ALU.mult)
                nc.vector.tensor_tensor(out=var[:], in0=v2[:], in1=var[:],
                                        op=ALU.subtract)
                nc.vector.tensor_scalar(out=var[:], in0=var[:], scalar1=1e-5,
                                        scalar2=None, op0=ALU.add)
                sd = sb.tile([P, 1], F32, tag="sd")
                nc.scalar.activation(sd[:], var[:], AF.Sqrt)
                rs = sb.tile([P, 1], F32, tag="rs")
                nc.vector.reciprocal(rs[:], sd[:])
                a_bn = sb.tile([P, 1], F32, tag="a_bn")
                nc.vector.tensor_tensor(out=a_bn[:], in0=bngc, in1=rs[:],
                                        op=ALU.mult)
                nb = sb.tile([P, 1], F32, tag="nb")
                nc.vector.tensor_tensor(out=nb[:], in0=mu[:], in1=a_bn[:],
                                        op=ALU.mult)
                b_bn = sb.tile([P, 1], F32, tag="b_bn")
                nc.vector.tensor_tensor(out=b_bn[:], in0=bnbc, in1=nb[:],
                                        op=ALU.subtract)

                # ---- B1: h_node from subgraph roots (h_pm partitions
                # 0/32/64/96 hold the roots of subgraphs w*4+0..3) ----
                rt0 = sb.tile([P, 2, H], BF16, tag="b1p0")
                nc.sync.dma_start(
                    rt0[:], h_pm[0, :, :].rearrange("(a b) h -> b a h", a=2))
                rt1 = sb.tile([P, 2, H], BF16, tag="b1p1")
                nc.sync.dma_start(
                    rt1[:], h_pm[32, :, :].rearrange("(a b) h -> b a h", a=2))
                rt2 = sb.tile([P, 2, H], BF16, tag="b1p2")
                nc.sync.dma_start(
                    rt2[:], h_pm[64, :, :].rearrange("(a b) h -> b a h", a=2))
                rt3 = sb.tile([P, 2, H], BF16, tag="b1p3")
                nc.sync.dma_start(
                    rt3[:], h_pm[96, :, :].rearrange("(a b) h -> b a h", a=2))
                se = sb.tile([P, 2, H], F32, tag="b1se")
                nc.vector.tensor_tensor(out=se[:], in0=rt0[:], in1=rt1[:],
                                        op=ALU.add)
                seh = sb.tile([P, 2, H], BF16, tag="b1seh")
                nc.vector.tensor_scalar(out=seh[:], in0=se[:], scalar1=0.5,
                                        scalar2=None, op0=ALU.mult)
                so = sb.tile([P, 2, H], F32, tag="b1so")
                nc.vector.tensor_tensor(out=so[:], in0=rt2[:], in1=rt3[:],
                                        op=ALU.add)
                soh = sb.tile([P, 2, H], BF16, tag="b1soh")
                nc.vector.tensor_scalar(out=soh[:], in0=so[:], scalar1=0.5,
                                        scalar2=None, op0=ALU.mult)
                nc.sync.dma_start(
                    hn_d[0::2, :].rearrange("(a b) h -> b a h", a=2), seh[:])
                nc.sync.dma_start(
                    hn_d[1::2, :].rearrange("(a b) h -> b a h", a=2), soh[:])
                nc.gpsimd.collective_compute(
                    "AllGather", ALU.bypass, replica_groups=RG,
                    ins=[hn_d[:].opt()], outs=[hnfull_d[:].opt()])

                # ---- B2: global aggregation (dst-owned, batched gather) ----
                aggn = psagg.tile([P, GW, H], F32, space="PSUM", tag="psagg")
                for k in range(NWIN_G):
                    ntk = NTG[k]
                    o0 = int(OFFG[k])
                    gath = gp.tile([P, ntk, P], BF16, tag="g_gath")
                    for j in range(ntk):
                        nc.gpsimd.indirect_dma_start(
                            out=gath[:, j, :], out_offset=None,
                            in_=hnfull_d[:],
                            in_offset=bass.IndirectOffsetOnAxis(
                                ap=gidx_sb[:, o0 + j:o0 + j + 1], axis=0))
                    gtohc = sb.tile([32, ntk * P], BF16, tag="g_toh")
                    nc.sync.dma_start(gtohc[:],
                                      gtoh[:, o0 * P:(o0 + ntk) * P])
                    gdstc = sb.tile([P, ntk * P], BF16, tag="g_dst")
                    nc.sync.dma_start(gdstc[:],
                                      gdstoh[:, o0 * P:(o0 + ntk) * P])
                    for b0 in range(0, ntk, 4):
                        bn = min(4, ntk - b0)
                        mps = ps.tile([P, GW, H], F32, space="PSUM",
                                      tag="psmsg")
                        for i in range(bn):
                            t = b0 + i
                            nc.tensor.matmul(out=mps[:, i, :],
                                             lhsT=gtohc[:, t * P:(t + 1) * P],
                                             rhs=bond_sb[:],
                                             start=True, stop=True)
                        ms = sb.tile([P, GW, H], F32, tag="g_ms")
                        nc.vector.tensor_tensor(
                            out=ms[:, :bn, :],
                            in0=gath[:, b0:b0 + bn, :],
                            in1=mps[:, :bn, :], op=ALU.add)
                        msgb = sb.tile([P, GW, H], BF16, tag="g_msg")
                        nc.scalar.activation(msgb[:, :bn, :],
                                             ms[:, :bn, :], AF.Relu)
                        for i in range(bn):
                            t = b0 + i
                            nc.tensor.matmul(out=aggn[:, k, :],
                                             lhsT=gdstc[:, t * P:(t + 1) * P],
                                             rhs=msgb[:, i, :],
                                             start=(b0 + i == 0),
                                             stop=(b0 + i == ntk - 1))

                # hlinN = (1+eps_g)*hn + aggN  (our 512 rows)
                for j in range(NODES // P):
                    hn = sb.tile([P, H], BF16, tag="b2h")
                    nc.sync.dma_start(hn[:], hn_d[j * P:(j + 1) * P, :])
                    t0 = sb.tile([P, H], F32, tag="b2t")
                    nc.vector.tensor_scalar(out=t0[:], in0=hn[:],
                                            scalar1=1.0 + eps_g[li],
                                            scalar2=None, op0=ALU.mult)
                    hl = sb.tile([P, H], BF16, tag="b2l")
                    nc.vector.tensor_tensor(out=hl[:], in0=aggn[:, j, :],
                                            in1=t0[:], op=ALU.add)
                    nc.sync.dma_start(hlinN_d[j * P:(j + 1) * P, :], hl[:])

                # global MLP on 512 rows (feature-major, one tile)
                hT = sb.tile([H, NODES], BF16, tag="n_hT")
                nc.sync.dma_start_transpose(hT[:], hlinN_d[:, :])
                wg1 = sb.tile([H, H], BF16, tag="wg1")
                nc.sync.dma_start(wg1[:], gw1[wof:wof + H, :])
                p1 = ps2.tile([H, NODES], F32, space="PSUM", tag="ps512")
                nc.tensor.matmul(out=p1[:], lhsT=wg1[:], rhs=hT[:],
                                 start=True, stop=True)
                mid = sb.tile([H, NODES], BF16, tag="n_mid")
                nc.scalar.activation(mid[:], p1[:], AF.Relu, bias=gb1c)
                wg2 = sb.tile([H, H], BF16, tag="wg2")
                nc.sync.dma_start(wg2[:], gw2[wof:wof + H, :])
                p2 = ps2.tile([H, NODES], F32, space="PSUM", tag="ps512")
                nc.tensor.matmul(out=p2[:], lhsT=wg2[:], rhs=mid[:],
                                 start=True, stop=True)
                hcr = sb.tile([H, NODES], BF16, tag="n_hcr")
                nc.scalar.activation(hcr[:], p2[:], AF.Relu, bias=gb2c)
                # BN-global stats (local slice) + allreduce
                stg = sb.tile([P, 2], F32, tag="stg")
                nc.vector.tensor_reduce(out=stg[:, 0:1], in_=hcr[:],
                                        axis=mybir.AxisListType.X, op=ALU.add)
                sqg_scr = sb.tile([H, NODES], F32, tag="n_sq")
                nc.scalar.activation(sqg_scr[:], hcr[:], AF.Square,
                                     accum_out=stg[:, 1:2])
                nc.sync.dma_start(statg_in[:], stg[:])
                nc.gpsimd.collective_compute(
                    "AllReduce", ALU.add, replica_groups=RG,
                    ins=[statg_in[:].opt()], outs=[statg_out[:].opt()])
                sgo = sb.tile([P, 2], F32, tag="sgo")
                nc.sync.dma_start(sgo[:], statg_out[:])
                mug = sb.tile([P, 1], F32, tag="mug")
                nc.vector.tensor_scalar(out=mug[:], in0=sgo[:, 0:1],
                                        scalar1=1.0 / Nn, scalar2=None,
                                        op0=ALU.mult)
                varg = sb.tile([P, 1], F32, tag="varg")
                nc.vector.tensor_tensor(out=varg[:], in0=mug[:], in1=mug[:],
                                        op=ALU.mult)
                v2g = sb.tile([P, 1], F32, tag="v2g")
                nc.vector.tensor_scalar(out=v2g[:], in0=sgo[:, 1:2],
                                        scalar1=1.0 / Nn, scalar2=None,
                                        op0=ALU.mult)
                nc.vector.tensor_tensor(out=varg[:], in0=v2g[:], in1=varg[:],
                                        op=ALU.subtract)
                nc.vector.tensor_scalar(out=varg[:], in0=varg[:], scalar1=1e-5,
                                        scalar2=None, op0=ALU.add)
                sdg = sb.tile([P, 1], F32, tag="sdg")
                nc.scalar.activation(sdg[:], varg[:], AF.Sqrt)
                rsg = sb.tile([P, 1], F32, tag="rsg")
                nc.vector.reciprocal(rsg[:], sdg[:])
                ag_bn = sb.tile([P, 1], F32, tag="ag_bn")
                nc.vector.tensor_tensor(out=ag_bn[:], in0=gbngc, in1=rsg[:],
                                        op=ALU.mult)
                nbg = sb.tile([P, 1], F32, tag="nbg")
                nc.vector.tensor_tensor(out=nbg[:], in0=mug[:], in1=ag_bn[:],
                                        op=ALU.mult)
                bg_bn = sb.tile([P, 1], F32, tag="bg_bn")
                nc.vector.tensor_tensor(out=bg_bn[:], in0=gbnbc, in1=nbg[:],
                                        op=ALU.subtract)
                # h_node_new^T = hn^T + BN(hcr)
                hnT = sb.tile([H, NODES], BF16, tag="n_hnT")
                nc.sync.dma_start_transpose(hnT[:], hn_d[:, :])
                hcb = sb.tile([H, NODES], F32, tag="n_hcb")
                nc.vector.tensor_scalar(out=hcb[:], in0=hcr[:],
                                        scalar1=ag_bn[:], scalar2=bg_bn[:],
                                        op0=ALU.mult, op1=ALU.add)
                hnn = sb.tile([H, NODES], BF16, tag="n_hnn")
                nc.vector.tensor_tensor(out=hnn[:], in0=hcb[:], in1=hnT[:],
                                        op=ALU.add)
                # hb^T = bcast_w^T @ hnn^T
                wbc = sb.tile([H, H], BF16, tag="wbc")
                nc.sync.dma_start(wbc[:], bcw[wof:wof + H, :])
                p3 = ps2.tile([H, NODES], F32, space="PSUM", tag="ps512")
                nc.tensor.matmul(out=p3[:], lhsT=wbc[:], rhs=hnn[:],
                                 start=True, stop=True)
                hbT = sbw.tile([H, NODES], BF16, tag="hbT")
                nc.vector.tensor_copy(out=hbT[:], in_=p3[:])

                # ---- B3/B4: cat MLP + LN + residual ----
                wc1t = sbw.tile([H, H], BF16, tag="wc1t")
                nc.sync.dma_start(wc1t[:], cw1t[wof:wof + H, :])
                wc1b = sbw.tile([H, H], BF16, tag="wc1b")
                nc.sync.dma_start(wc1b[:], cw1b[wof:wof + H, :])
                wc2 = sbw.tile([H, H], BF16, tag="wc2")
                nc.sync.dma_start(wc2[:], cw2[wof:wof + H, :])
                cb2 = sbw.tile([P, H], F32, tag="cb2")
                nc.sync.dma_start(cb2[:], cb2rep[li * P:(li + 1) * P, :])
                lng = sbw.tile([P, H], F32, tag="lng")
                nc.sync.dma_start(lng[:], lngrep[li * P:(li + 1) * P, :])
                lnb = sbw.tile([P, H], F32, tag="lnb")
                nc.sync.dma_start(lnb[:], lnbrep[li * P:(li + 1) * P, :])
                for rt in range(ROWS // 512):
                    hrt = sb.tile([H, 512], BF16, tag="c_hrt")
                    nc.sync.dma_start(hrt[:],
                                      hrT_d[:, rt * 512:(rt + 1) * 512])
                    hbn = sb.tile([H, 512], BF16, tag="c_hbn")
                    nc.vector.tensor_scalar(out=hbn[:], in0=hrt[:],
                                            scalar1=a_bn[:], scalar2=b_bn[:],
                                            op0=ALU.mult, op1=ALU.add)
                    pc = ps2.tile([H, 512], F32, space="PSUM", tag="ps512")
                    nc.tensor.matmul(out=pc[:], lhsT=wc1t[:], rhs=hbn[:],
                                     start=True, stop=False)
                    hbe = hbT[:, rt * 8:(rt + 1) * 8, None]
                    nc.tensor.matmul(out=pc[:], lhsT=wc1b[:],
                                     rhs=hbe.to_broadcast([H, 8, 64]),
                                     start=False, stop=True)
                    mid2 = sb.tile([H, 512], BF16, tag="c_mid2")
                    nc.scalar.activation(mid2[:], pc[:], AF.Gelu, bias=catb1c)
                    pn = ps2.tile([P, 4, H], F32, space="PSUM", tag="ps512")
                    for j in range(4):
                        nc.tensor.matmul(out=pn[:, j, :],
                                         lhsT=mid2[:, j * P:(j + 1) * P],
                                         rhs=wc2[:], start=True, stop=True)
                    xn = sb.tile([P, 4, H], F32, tag="c_xn")
                    nc.vector.tensor_tensor(
                        out=xn[:], in0=pn[:],
                        in1=cb2[:, None, :].to_broadcast([P, 4, H]),
                        op=ALU.add)
                    mu4 = sb.tile([P, 4], F32, tag="c_mu4")
                    nc.vector.tensor_reduce(out=mu4[:], in_=xn[:],
                                            axis=mybir.AxisListType.X,
                                            op=ALU.add)
                    nc.vector.tensor_scalar(out=mu4[:], in0=mu4[:],
                                            scalar1=1.0 / H, scalar2=None,
                                            op0=ALU.mult)
                    sq4 = sb.tile([P, 4, H], F32, tag="c_sq4")
                    nc.vector.tensor_tensor(out=sq4[:], in0=xn[:], in1=xn[:],
                                            op=ALU.mult)
                    s24 = sb.tile([P, 4], F32, tag="c_s24")
                    nc.vector.tensor_reduce(out=s24[:], in_=sq4[:],
                                            axis=mybir.AxisListType.X,
                                            op=ALU.add)
                    nc.vector.tensor_scalar(out=s24[:], in0=s24[:],
                                            scalar1=1.0 / H, scalar2=None,
                                            op0=ALU.mult)
                    m2 = sb.tile([P, 4], F32, tag="c_m2")
                    nc.vector.tensor_tensor(out=m2[:], in0=mu4[:], in1=mu4[:],
                                            op=ALU.mult)
                    nc.vector.tensor_tensor(out=s24[:], in0=s24[:], in1=m2[:],
                                            op=ALU.subtract)
                    nc.vector.tensor_scalar(out=s24[:], in0=s24[:],
                                            scalar1=1e-5, scalar2=None,
                                            op0=ALU.add)
                    sd4 = sb.tile([P, 4], F32, tag="c_sd4")
                    nc.scalar.activation(sd4[:], s24[:], AF.Sqrt)
                    rs4 = sb.tile([P, 4], F32, tag="c_rs4")
                    nc.vector.reciprocal(rs4[:], sd4[:])
                    nc.vector.tensor_tensor(
                        out=xn[:], in0=xn[:],
                        in1=mu4[:, :, None].to_broadcast([P, 4, H]),
                        op=ALU.subtract)
                    nc.vector.tensor_tensor(
                        out=xn[:], in0=xn[:],
                        in1=rs4[:, :, None].to_broadcast([P, 4, H]),
                        op=ALU.mult)
                    nc.vector.tensor_tensor(
                        out=xn[:], in0=xn[:],
                        in1=lng[:, None, :].to_broadcast([P, 4, H]),
                        op=ALU.mult)
                    nc.vector.tensor_tensor(
                        out=xn[:], in0=xn[:],
                        in1=lnb[:, None, :].to_broadcast([P, 4, H]),
                        op=ALU.add)
                    hin = sb.tile([P, 4, H], BF16, tag="c_hin")
                    nc.sync.dma_start(hin[:],
                                      h_pm[:, rt * 4:(rt + 1) * 4, :])
                    nc.vector.tensor_tensor(out=xn[:], in0=xn[:], in1=hin[:],
                                            op=ALU.add)
                    hout = sb.tile([P, 4, H], BF16, tag="c_hout")
                    vsl = validf_sb[:, rt * 4:(rt + 1) * 4, None]
                    nc.vector.tensor_tensor(
                        out=hout[:], in0=xn[:],
                        in1=vsl.to_broadcast([P, 4, H]), op=ALU.mult)
                    nc.sync.dma_start(h_pm[:, rt * 4:(rt + 1) * 4, :],
                                      hout[:])

            # ================= pooling =================
            pp = pspool.tile([NG, H], F32, space="PSUM", tag="poolps")
            for g in range(NWIN // GW):
                htile = sb.tile([P, GW, H], BF16, tag="p_h")
                nc.sync.dma_start(htile[:], h_pm[:, g * GW:(g + 1) * GW, :])
                wp = sb.tile([P, GW * NG], BF16, tag="p_w")
                nc.sync.dma_start(wp[:],
                                  wpool[:, g * GW * NG:(g + 1) * GW * NG])
                for k in range(GW):
                    rt = g * GW + k
                    nc.tensor.matmul(out=pp[:],
                                     lhsT=wp[:, k * NG:(k + 1) * NG],
                                     rhs=htile[:, k, :],
                                     start=(rt == 0), stop=(rt == NWIN - 1))
            po = sb.tile([NG, H], F32, tag="p_o")
            nc.vector.tensor_copy(out=po[:], in_=pp[:])
            nc.sync.dma_start(pool_in[:], po[:])
            nc.gpsimd.collective_compute(
                "AllReduce", ALU.add, replica_groups=RG,
                ins=[pool_in[:].opt()], outs=[pool_out[:].opt()])
            fo = sb.tile([NG, H], F32, tag="p_f")
            nc.sync.dma_start(fo[:], pool_out[:])
            nc.sync.dma_start(out_ext[:], fo[:])

    nc.finalize()
    return nc


# ----------------------------------------------------------------------------
# kernel entry
# ----------------------------------------------------------------------------

def kernel(**inputs):
    atom_emb = np.asarray(inputs["atom_emb"], np.float32)
    bond_emb = np.asarray(inputs["bond_emb"], np.float32)
    rwse_w = np.asarray(inputs["rwse_w"], np.float32)
    rwse_b = np.asarray(inputs["rwse_b"], np.float32)
    rwse = np.asarray(inputs["rwse"], np.float32)
    l_eps = np.asarray(inputs["l_eps"], np.float32)
    l_w1 = np.asarray(inputs["l_w1"], np.float32)
    l_b1 = np.asarray(inputs["l_b1"], np.float32)
    l_w2 = np.asarray(inputs["l_w2"], np.float32)
    l_b2 = np.asarray(inputs["l_b2"], np.float32)
    l_bng = np.asarray(inputs["l_bng"], np.float32)
    l_bnb = np.asarray(inputs["l_bnb"], np.float32)
    g_eps = np.asarray(inputs["g_eps"], np.float32)
    g_w1 = np.asarray(inputs["g_w1"], np.float32)
    g_b1 = np.asarray(inputs["g_b1"], np.float32)
    g_w2 = np.asarray(inputs["g_w2"], np.float32)
    g_b2 = np.asarray(inputs["g_b2"], np.float32)
    g_bng = np.asarray(inputs["g_bng"], np.float32)
    g_bnb = np.asarray(inputs["g_bnb"], np.float32)
    bcast_w = np.asarray(inputs["bcast_w"], np.float32)
    cat_w1 = np.asarray(inputs["cat_w1"], np.float32)
    cat_b1 = np.asarray(inputs["cat_b1"], np.float32)
    cat_w2 = np.asarray(inputs["cat_w2"], np.float32)
    cat_b2 = np.asarray(inputs["cat_b2"], np.float32)
    ln_g = np.asarray(inputs["ln_g"], np.float32)
    ln_b = np.asarray(inputs["ln_b"], np.float32)
    x_ids = np.asarray(inputs["x_ids"], np.int64)
    intra_ei = np.asarray(inputs["intra_ei"], np.int64)
    intra_ea_ids = np.asarray(inputs["intra_ea_ids"], np.int64)
    global_ei = np.asarray(inputs["global_ei"], np.int64)
    global_ea_ids = np.asarray(inputs["global_ea_ids"], np.int64)
    node_ids = np.asarray(inputs["node_ids"], np.int64)
    valid = np.asarray(inputs["valid"], np.int64)
    batch = np.asarray(inputs["batch"], np.int64)

    bond_ext = np.zeros((32, H), np.float32)
    bond_ext[:16] = bond_emb
    rww = np.zeros((32, H), np.float32)
    rww[:16] = rwse_w
    rww[16] = rwse_b

    # ---- per-core intra edge split + tile counts ----
    esrc, edst = intra_ei[0], intra_ei[1]
    ecore = (edst // K_SUB) // SUBS
    per_core = []
    win_counts = []
    for c in range(NCORES):
        m = ecore == c
        s = esrc[m] - c * ROWS
        d = edst[m] - c * ROWS
        t = intra_ea_ids[m]
        per_core.append((s, d, t))
        win_counts.append(np.bincount(d // P, minlength=NWIN))
    NT = np.ones(NWIN, np.int64)
    for cnt in win_counts:
        NT = np.maximum(NT, (cnt + P - 1) // P)
    OFF = np.concatenate([[0], np.cumsum(NT)]).astype(np.int64)

    # ---- per-core global edge split (dst-window-owned) ----
    gsrc_, gdst_ = global_ei[0], global_ei[1]
    gcore = gdst_ // NODES
    gper_core = []
    gwin_counts = []
    for c in range(NCORES):
        m = gcore == c
        gper_core.append((gsrc_[m], gdst_[m], global_ea_ids[m]))
        gwin_counts.append(
            np.bincount((gdst_[m] - c * NODES) // P, minlength=NWIN_G))
    NTG = np.ones(NWIN_G, np.int64)
    for cnt in gwin_counts:
        NTG = np.maximum(NTG, (cnt + P - 1) // P)
    OFFG = np.concatenate([[0], np.cumsum(NTG)]).astype(np.int64)

    # ---- pooling weights per core ----
    valid_f = valid.astype(np.float32)
    cnt_s = valid_f.reshape(S_TOT, K_SUB).sum(1)
    wrow = 1.0 / (2.0 * np.maximum(cnt_s, 1.0))       # per subgraph
    node_of_sub = np.arange(S_TOT) // M_SUB
    graph_of_sub = batch[node_of_sub]                  # [S_TOT]

    nc = build_program(list(NT), list(NTG), [float(x) for x in l_eps],
                       [float(x) for x in g_eps])

    bias_cols = np.zeros((P, 8 * L), np.float32)
    bias2_cols = np.zeros((P, 4 * L), np.float32)
    for li in range(L):
        bias_cols[:, 8 * li + 0] = l_b1[li]
        bias_cols[:, 8 * li + 1] = l_b2[li]
        bias_cols[:, 8 * li + 2] = g_b1[li]
        bias_cols[:, 8 * li + 3] = g_b2[li]
        bias_cols[:, 8 * li + 4] = cat_b1[li]
        bias_cols[:, 8 * li + 5] = l_bng[li]
        bias_cols[:, 8 * li + 6] = l_bnb[li]
        bias_cols[:, 8 * li + 7] = g_bng[li]
        bias2_cols[:, 4 * li] = g_bnb[li]

    in_maps = []
    for c in range(NCORES):
        r0 = c * ROWS
        d = {}
        d["atom"] = atom_emb.astype(BF)
        d["bond"] = bond_ext.astype(BF)
        d["rww"] = rww.astype(BF)
        # h0 tables (valid-masked)
        vloc = valid_f[r0:r0 + ROWS]
        xids = x_ids[r0:r0 + ROWS]
        nids = node_ids[r0:r0 + ROWS]
        xoh = np.zeros((P, ROWS), BF)
        vmask = vloc > 0
        cols = np.arange(ROWS)
        xoh[xids[vmask], cols[vmask]] = 1.0
        rw17 = np.zeros((32, ROWS), np.float32)
        rw17[:16, vmask] = rwse[nids[vmask]].T
        rw17[16, vmask] = 1.0
        d["xoh"] = xoh
        d["rw17"] = rw17.astype(BF)
        # intra tables ([selT | dstoh] interleaved per tile)
        s_, d_, t_ = per_core[c]
        selT, toh_t, dstoh_t = build_intra_tables(s_, d_, t_, NT, OFF)
        TTc = selT.shape[1] // P
        tabs = np.empty((P, TTc, 2, P), BF)
        tabs[:, :, 0, :] = selT.reshape(P, TTc, P)
        tabs[:, :, 1, :] = dstoh_t.reshape(P, TTc, P)
        d["tabs"] = np.ascontiguousarray(tabs.reshape(P, TTc * 2 * P))
        d["tohi"] = toh_t
        # global tables
        gs_, gd_, gt_ = gper_core[c]
        gsrc_flat, gtoh_t, gdstoh_t = build_global_tables(
            gs_, gd_, gt_, NTG, OFFG, c * NODES)
        d["gtoh"] = gtoh_t
        d["gdstoh"] = gdstoh_t
        d["gidx"] = np.ascontiguousarray(
            gsrc_flat.reshape(-1, P).T).astype(np.int32)
        d["wl1"] = l_w1.reshape(L * H, H).astype(BF)
        d["wl2"] = l_w2.reshape(L * H, H).astype(BF)
        d["gw1"] = g_w1.reshape(L * H, H).astype(BF)
        d["gw2"] = g_w2.reshape(L * H, H).astype(BF)
        d["bcw"] = bcast_w.reshape(L * H, H).astype(BF)
        d["cw1t"] = cat_w1[:, :H, :].reshape(L * H, H).astype(BF)
        d["cw1b"] = cat_w1[:, H:, :].reshape(L * H, H).astype(BF)
        d["cw2"] = cat_w2.reshape(L * H, H).astype(BF)
        d["bias_cols"] = bias_cols
        d["bias2_cols"] = bias2_cols
        d["cb2rep"] = np.repeat(cat_b2[:, None, :], P, 1).reshape(L * P, H).astype(np.float32)
        d["lngrep"] = np.repeat(ln_g[:, None, :], P, 1).reshape(L * P, H).astype(np.float32)
        d["lnbrep"] = np.repeat(ln_b[:, None, :], P, 1).reshape(L * P, H).astype(np.float32)
        d["validf"] = np.ascontiguousarray(vloc.reshape(NWIN, P).T)
        wp = np.zeros((ROWS, NG), np.float32)
        for s in range(SUBS):
            gs = c * SUBS + s
            wp[s * K_SUB:(s + 1) * K_SUB, graph_of_sub[gs]] = wrow[gs]
        d["wpool"] = np.ascontiguousarray(
            wp.reshape(NWIN, P, NG).transpose(1, 0, 2).reshape(P, NWIN * NG)).astype(BF)
        in_maps.append(d)

    if os.environ.get("BASS_KERNEL_SIM"):
        from concourse.bass_interp import MultiCoreSim
        sim = MultiCoreSim(nc, num_cores=NCORES,
                           num_workers=int(os.environ.get("BASS_SIM_WORKERS", "8")))
        for c in range(NCORES):
            cs = sim.cores.get(c)
            if cs is None:
                continue
            for name, val in in_maps[c].items():
                view = cs.tensor(name)
                view[:] = val
        sim.simulate()
        out = sim.cores[0].tensor("out") if 0 in sim.cores else sim.outs[0]["out"]
        kernel.last_exec_ns = None
        return np.asarray(out, np.float32)

    res = run_bass_kernel_spmd(nc, in_maps, list(range(NCORES)),
                               **_extra_run_kwargs())
    out = res.results[0]["out"]
    kernel.last_exec_ns = res.exec_time_ns
    return np.asarray(out, np.float32)


def _extra_run_kwargs():
    kw = {}
    if os.environ.get("BASS_KERNEL_TRACE"):
        kw["trace"] = True
    return kw


kernel.last_exec_ns = None
